# revision 4
# baseline (speedup 1.0000x reference)
"""Kinematics LSTM decoder on 8 trn2 NeuronCores — wire-optimized v2.

The axon tunnel moves host->device bytes at ~40MB/s, so the per-call
wall is dominated by input transfer. v2 ships the LSTM weights as int8
(dequantized on device into SBUF-resident f32 tiles; quant sim l2rel
2.5e-3 vs 2e-2 budget) and computes the encoder means host-side, cutting
the payload from ~186MB to ~41MB.

Device strategy (unchanged from v1): model-parallel over the 4608 gate
dim (576 gate cols / core = 144 h cols / core). Recurrence: 25 steps x 6
cells; per-cell AllGather of the transposed h slice through DRAM bounce
buffers. Gates layout [batch, gatecols], per-core col order [i f | o g];
matmuls in f32r. Layers 2,3 share weights AND input -> batch-stacked.
"""
import numpy as np

B, T_ENC, D_IN, H, T_OUT = 64, 49, 54, 1152, 25
NC_ = 8          # cores
HS = H // NC_    # 144 h cols per core
GS = 4 * HS      # 576 gate cols per core
NK = H // 128    # 9 contraction chunks

PERM = [0, 1, 3, 2]  # pytorch gate order (i,f,g,o) -> per-core col order (i,f,o,g)
TAGS = [("0h", "Whh0"), ("1x", "Wih1"), ("1h", "Whh1"),
        ("Ax", "WihA"), ("Ah", "WhhA"), ("Lx", "WihL"), ("Lh", "WhhL")]
TAGIDX = {"0": 0, "1": 1, "A": 2, "L": 3}

_NF = NK * GS
_NH = _NF // 2
_NL = _NF // 4
_N8 = _NF // 8

# bits per big-weight tensor, in TAGS order (0h,1x,1h at 6; A/L cells at 5 —
# real-input sim: l2rel 7.9e-3, maxrel 9.7e-3 vs the 2e-2 gate)
TBITS = [6, 6, 6, 5, 5, 5, 5]
N6 = sum(1 for b in TBITS if b == 6)
N5 = sum(1 for b in TBITS if b == 5)


def _blob_layout():
    layout = [
        ("wscale", 128 * 24 * 4),
        ("bias", 4 * GS * 4),
        ("hb1", D_IN * 4),
        ("h0T", HS * B * 4),
        ("h1T", HS * B * 4),
        ("cin", B * HS * 4),
        ("p0", B * D_IN * 4),
        ("wh8", NK * 128 * 54),
        ("w0x8", 54 * GS),
        ("wqh", 7 * 128 * _NH),
        ("wq6l", N6 * 128 * _NL),
        ("wq5l", N5 * 128 * _N8),
    ]
    off, d = 0, {}
    for name, nb in layout:
        d[name] = (off, nb)
        off += nb
    return d, off


BLOB_OFF, BLOB_BYTES = _blob_layout()

_compiled = None


def _enable_jax_cache():
    """Persistent XLA executable cache: without it every
    run_bass_kernel_spmd call re-lowers + re-runs the walrus NEFF
    compile (~0.9s/call)."""
    try:
        import jax
        jax.config.update("jax_compilation_cache_dir", "/tmp/bass_jax_cache")
        jax.config.update("jax_persistent_cache_min_entry_size_bytes", -1)
        jax.config.update("jax_persistent_cache_min_compile_time_secs", 0)
    except Exception:
        pass


_enable_jax_cache()



def _build():
    import concourse.bass as bass
    import concourse.bacc as bacc
    import concourse.tile as tile
    import concourse.mybir as mybir

    f32 = mybir.dt.float32
    f32r = mybir.dt.float32r
    bf16 = mybir.dt.bfloat16
    i8 = mybir.dt.int8
    u8 = mybir.dt.uint8
    AF = mybir.ActivationFunctionType
    OP = mybir.AluOpType

    NF = NK * GS          # 5184 flat weight cols per partition
    NH = NF // 2          # 2592
    NL = NF // 4          # 1296

    nc = bacc.Bacc("TRN2", target_bir_lowering=False, debug=False,
                   num_devices=NC_)

    # single per-core input blob; section offsets must match _prep_inputs
    blob = nc.dram_tensor("blob", [BLOB_BYTES], u8, kind="ExternalInput")

    def bsec(name, dt_):
        off, nbytes = BLOB_OFF[name]
        ap = blob[off:off + nbytes]
        return ap if dt_ == u8 else ap.bitcast(dt_)

    out_d = nc.dram_tensor("out", [B, T_OUT, D_IN], bf16, kind="ExternalOutput")

    RG = [list(range(NC_))]

    with tile.TileContext(nc) as tc:
        with tc.tile_pool(name="wpool", bufs=1) as wp, \
             tc.tile_pool(name="stg", bufs=1) as stg, \
             tc.tile_pool(name="state", bufs=1) as st, \
             tc.tile_pool(name="work", bufs=3) as wk, \
             tc.tile_pool(name="hnewp", bufs=2) as hp, \
             tc.tile_pool(name="psg", bufs=2, space="PSUM") as psg, \
             tc.tile_pool(name="pst", bufs=2, space="PSUM") as pst, \
             tc.tile_pool(name="psh", bufs=1, space="PSUM") as psh, \
             tc.tile_pool(name="dram", bufs=6, space="DRAM") as dp:

            # ---- scales / misc constants ----
            wsc = wp.tile([128, 24], f32, tag="wsc", name="wsc")
            nc.sync.dma_start(
                wsc[:], bsec("wscale", f32).rearrange("(p f) -> p f", p=128))
            ones = wp.tile([1, 128], f32, tag="ones", name="ones")
            nc.vector.memset(ones[:], 1.0)
            ones128 = wp.tile([128, 128], f32, tag="ones128", name="ones128")
            nc.vector.memset(ones128[:], 1.0)
            ident = wp.tile([128, 128], f32, tag="ident", name="ident")
            nc.gpsimd.affine_select(ident[:], ones128[:], pattern=[[-1, 128]],
                                    compare_op=OP.is_equal, fill=0.0,
                                    base=0, channel_multiplier=1)
            hb1 = wp.tile([1, D_IN], f32, tag="hb1", name="hb1")
            nc.sync.dma_start(
                hb1[:], bsec("hb1", f32).rearrange("(p f) -> p f", p=1))
            boff = BLOB_OFF["bias"][0]
            b_sb = {}
            for ti, tg in enumerate("01AL"):
                b_sb[tg] = wp.tile([1, GS], f32, tag="b" + tg, name="b" + tg)
                nc.sync.dma_start(
                    b_sb[tg][:],
                    blob[boff + ti * GS * 4:boff + (ti + 1) * GS * 4]
                    .bitcast(f32).rearrange("(p f) -> p f", p=1))

            # ---- unpack int6/int5 weights into SBUF-resident f32r tiles ----
            # int6: v = round(w/s)+32 in [1,63]; h=v>>2 (4b), l=v&3 (2b);
            #       w = h*(4s) - 32s + l*s
            # int5: v = round(w/s)+16 in [1,31]; h=v>>1 (4b), l=v&1 (1b);
            #       w = h*(2s) - 16s + l*s
            N8 = NF // 8
            w_sb = {}
            hoff = BLOB_OFF["wqh"][0]
            l6off = BLOB_OFF["wq6l"][0]
            l5off = BLOB_OFF["wq5l"][0]
            i6 = i5 = 0
            for i, (tag, _) in enumerate(TAGS):
                bits = TBITS[i]
                sh = stg.tile([128, NH], u8, tag="sth", name="sth" + tag)
                nc.sync.dma_start(
                    sh[:], blob[hoff + i * 128 * NH:hoff + (i + 1) * 128 * NH]
                    .rearrange("(p f) -> p f", p=128))
                if bits == 6:
                    lsz = NL
                    loff = l6off + i6 * 128 * NL
                    i6 += 1
                else:
                    lsz = N8
                    loff = l5off + i5 * 128 * N8
                    i5 += 1
                sl = stg.tile([128, lsz], u8, tag=f"stl{bits}", name="stl" + tag)
                nc.sync.dma_start(
                    sl[:], blob[loff:loff + 128 * lsz]
                    .rearrange("(p f) -> p f", p=128))
                w_sb[tag] = wp.tile([128, NK, GS], f32r, tag="w" + tag, name="w" + tag)
                wf = w_sb[tag][:].rearrange("p a b -> p (a b)")
                shi = wsc[:, 3 * i:3 * i + 1]
                soff = wsc[:, 3 * i + 1:3 * i + 2]
                s1 = wsc[:, 3 * i + 2:3 * i + 3]
                # hi plane (4 bits either way)
                hsh = 2 if bits == 6 else 1
                th = stg.tile([128, NH], u8, tag="th", name="th" + tag)
                nc.vector.tensor_scalar(th[:], sh[:], 15, None, op0=OP.bitwise_and)
                nc.vector.tensor_scalar(wf[:, 0:NH], th[:], shi, soff,
                                        op0=OP.mult, op1=OP.add)
                nc.vector.tensor_scalar(th[:], sh[:], 4, None,
                                        op0=OP.logical_shift_right)
                nc.vector.tensor_scalar(wf[:, NH:NF], th[:], shi, soff,
                                        op0=OP.mult, op1=OP.add)
                # lo plane: 8 subranges of N8 columns each
                for k in range(8):
                    if bits == 6:
                        qd, hh = k // 2, k % 2
                        lsrc = sl[:, hh * N8:(hh + 1) * N8]
                        shift, mask, last = 2 * qd, 3, (qd == 3)
                    else:
                        lsrc = sl[:]
                        shift, mask, last = k, 1, (k == 7)
                    dst = wf[:, k * N8:(k + 1) * N8] if bits == 5 else \
                        wf[:, (k // 2) * NL + (k % 2) * N8:
                           (k // 2) * NL + (k % 2 + 1) * N8]
                    tl = stg.tile([128, N8], u8, tag="tl", name="tl")
                    if shift == 0:
                        nc.vector.tensor_scalar(tl[:], lsrc, mask, None,
                                                op0=OP.bitwise_and)
                    elif last:
                        nc.vector.tensor_scalar(tl[:], lsrc, shift, None,
                                                op0=OP.logical_shift_right)
                    else:
                        nc.vector.tensor_scalar(tl[:], lsrc, shift, mask,
                                                op0=OP.logical_shift_right,
                                                op1=OP.bitwise_and)
                    tf = stg.tile([128, N8], f32, tag="tf", name="tf")
                    nc.vector.tensor_scalar(tf[:], tl[:], s1, None,
                                            op0=OP.mult)
                    nc.vector.tensor_tensor(dst, dst, tf[:], op=OP.add)

            sg0 = stg.tile([54, GS], i8, tag="stg0", name="stg0x")
            nc.sync.dma_start(
                sg0[:], bsec("w0x8", i8).rearrange("(p f) -> p f", p=54))
            w_sb["0x"] = wp.tile([54, GS], f32r, tag="w0x", name="w0x")
            nc.vector.tensor_scalar(w_sb["0x"][:], sg0[:], wsc[0:54, 21:22],
                                    None, op0=OP.mult)
            sgh = stg.tile([128, NK, 54], i8, tag="stgh", name="stgh")
            nc.sync.dma_start(
                sgh[:], bsec("wh8", i8).rearrange("(c k n) -> k c n",
                                                  c=NK, k=128, n=54))
            wh_all = wp.tile([128, NK, 54], f32, tag="whall", name="whall")
            nc.vector.tensor_scalar(wh_all[:], sgh[:], wsc[:, 22:23],
                                    None, op0=OP.mult)

            # ---- persistent state ----
            hT0 = st.tile([128, NK, B], f32r, tag="hT0", name="hT0")        # h0.T
            hT1 = st.tile([128, NK, 2 * B], f32r, tag="hT1", name="hT1")    # h1.T duplicated
            hTA = st.tile([128, NK, 2 * B], f32r, tag="hTA", name="hTA")    # h2.T | h3.T
            hTL = st.tile([128, NK, 2 * B], f32r, tag="hTL", name="hTL")    # h4.T | h5.T
            c_st = {0: st.tile([B, HS], f32, tag="c0", name="c0"),
                    1: st.tile([B, HS], f32, tag="c1", name="c1"),
                    "A": st.tile([2 * B, HS], f32, tag="cA", name="cA"),
                    4: st.tile([B, HS], f32, tag="c4", name="c4"),
                    5: st.tile([B, HS], f32, tag="c5", name="c5")}
            x0b = st.tile([B, D_IN], f32, tag="x0b", name="x0b")
            x0T = st.tile([D_IN, B], f32r, tag="x0T", name="x0T")

            zf = wp.tile([128, 2 * B], f32, tag="zf", name="zf")
            nc.vector.memset(zf[:], 0.0)
            for c in range(NK):
                nc.scalar.copy(hTA[:, c, :], zf[:])
                nc.scalar.copy(hTL[:, c, :], zf[:])
            nc.vector.memset(c_st["A"][:], 0.0)
            nc.vector.memset(c_st[4][:], 0.0)
            nc.vector.memset(c_st[5][:], 0.0)

            r32 = lambda ap: ap.bitcast(f32r)

            def transpose_to(dst_dram_slices, src_sb, rows, cols):
                """src_sb [rows<=128, cols] -> transposed [cols, rows] written
                to dram col-splits."""
                done = 0
                while done < cols:
                    n = min(128, cols - done)
                    pt = pst.tile([128, 128], f32, tag="pt", name="pt")
                    nc.tensor.transpose(pt[0:n, 0:rows],
                                        src_sb[0:rows, done:done + n],
                                        ident[0:rows, 0:rows])
                    cp = wk.tile([128, 128], f32r, tag="tcp", name="tcp")
                    nc.scalar.copy(cp[0:n, 0:rows], pt[0:n, 0:rows])
                    for (dap, lo, hi) in dst_dram_slices:
                        nc.sync.dma_start(dap[done:done + n, :],
                                          cp[0:n, lo:hi])
                    done += n

            def allgather(n_rows):
                gin = dp.tile([n_rows, B], f32r, tag="agin", name="agin")
                gout = dp.tile([NC_ * n_rows, B], f32r, tag="agout", name="agout")
                return gin, gout

            def do_ag(gin, gout):
                nc.gpsimd.collective_compute(
                    "AllGather", OP.bypass, replica_groups=RG,
                    ins=[gin[:].opt()], outs=[gout[:].opt()])

            def dma_back(gout, dst, lo, hi):
                nc.sync.dma_start(
                    dst[:, :, lo:hi],
                    gout[:].rearrange("(c k) n -> k c n", k=128))

            # ---- prologue: states from host-computed means ----
            nc.sync.dma_start(
                c_st[0][:], bsec("cin", f32).rearrange("(p f) -> p f", p=B))
            nc.vector.tensor_copy(c_st[1][:], c_st[0][:])

            for (nm, dsts) in (("h0T", [(hT0, 0, B)]),
                               ("h1T", [(hT1, 0, B), (hT1, B, 2 * B)])):
                gin, gout = allgather(HS)
                nc.sync.dma_start(
                    gin[:], bsec(nm, f32r).rearrange("(p f) -> p f", p=HS))
                do_ag(gin, gout)
                for (dst, lo, hi) in dsts:
                    dma_back(gout, dst, lo, hi)

            # x0
            nc.sync.dma_start(
                x0b[:], bsec("p0", f32).rearrange("(p f) -> p f", p=B))
            ptp = pst.tile([128, 128], f32, tag="pt", name="pt")
            nc.tensor.transpose(ptp[0:D_IN, 0:B], x0b[0:B, 0:D_IN],
                                ident[0:B, 0:B])
            nc.scalar.copy(x0T[:], ptp[0:D_IN, 0:B])

            # ---- helpers for the recurrence ----
            def gate_mms(g0, g1, rows, wtag, x_chunks, h_chunks):
                first = True
                for (lhsT, wkey, c) in h_chunks + x_chunks:
                    if wkey == "0x":
                        r0 = w_sb["0x"][0:54, 0:288]
                        r1 = w_sb["0x"][0:54, 288:GS]
                    else:
                        r0 = w_sb[wkey][:, c, 0:288]
                        r1 = w_sb[wkey][:, c, 288:GS]
                    nc.tensor.matmul(g0[0:rows, :], r32(lhsT), r32(r0),
                                     start=first, stop=False)
                    nc.tensor.matmul(g1[0:rows, :], r32(lhsT), r32(r1),
                                     start=first, stop=False)
                    first = False
                nc.tensor.matmul(g0[0:rows, :], ones[0:1, 0:rows],
                                 b_sb[wtag][0:1, 0:288],
                                 start=False, stop=True)
                nc.tensor.matmul(g1[0:rows, :], ones[0:1, 0:rows],
                                 b_sb[wtag][0:1, 288:GS],
                                 start=False, stop=True)

            def elementwise(g0, g1, rows, c_tile, crange):
                """gates [i f | o g]; returns h_new sbuf tile [rows, HS]"""
                sif = wk.tile([128, 2 * HS], f32, tag="sif", name="sif")
                nc.scalar.activation(sif[0:rows, :], g0[0:rows, :], AF.Sigmoid)
                so = wk.tile([128, HS], f32, tag="so", name="so")
                nc.scalar.activation(so[0:rows, :], g1[0:rows, 0:HS], AF.Sigmoid)
                tg = wk.tile([128, HS], f32, tag="tg", name="tg")
                nc.scalar.activation(tg[0:rows, :], g1[0:rows, HS:2 * HS], AF.Tanh)
                t1 = wk.tile([128, HS], f32, tag="t1", name="t1")
                cs = c_tile[crange[0]:crange[1], :]
                nc.vector.tensor_tensor(t1[0:rows, :], sif[0:rows, HS:2 * HS],
                                        cs, op=OP.mult)
                t2 = wk.tile([128, HS], f32, tag="t2", name="t2")
                nc.vector.tensor_tensor(t2[0:rows, :], sif[0:rows, 0:HS],
                                        tg[0:rows, :], op=OP.mult)
                nc.vector.tensor_tensor(cs, t1[0:rows, :], t2[0:rows, :],
                                        op=OP.add)
                tc_ = wk.tile([128, HS], f32, tag="tc", name="tc")
                nc.scalar.activation(tc_[0:rows, :], cs, AF.Tanh)
                hn = hp.tile([128, HS], f32, tag="hnew", name="hnew")
                nc.vector.tensor_tensor(hn[0:rows, :], so[0:rows, :],
                                        tc_[0:rows, :], op=OP.mult)
                return hn

            # ---- recurrence ----
            for t in range(T_OUT):
                # L0
                g0 = psg.tile([128, 288], f32, tag="g0", name="g0")
                g1 = psg.tile([128, 288], f32, tag="g1", name="g1")
                gate_mms(g0, g1, B, "0",
                         x_chunks=[(x0T[0:54, 0:B], "0x", 0)],
                         h_chunks=[(hT0[:, c, :], "0h", c) for c in range(NK)])
                hn0 = elementwise(g0, g1, B, c_st[0], (0, B))
                gin0, gout0 = allgather(HS)
                transpose_to([(gin0[:], 0, B)], hn0, B, HS)
                do_ag(gin0, gout0)
                dma_back(gout0, hT0, 0, B)

                # L1 (x = new h0)
                g0 = psg.tile([128, 288], f32, tag="g0", name="g0")
                g1 = psg.tile([128, 288], f32, tag="g1", name="g1")
                gate_mms(g0, g1, B, "1",
                         x_chunks=[(hT0[:, c, :], "1x", c) for c in range(NK)],
                         h_chunks=[(hT1[:, c, 0:B], "1h", c) for c in range(NK)])
                hn1 = elementwise(g0, g1, B, c_st[1], (0, B))
                gin1, gout1 = allgather(HS)
                transpose_to([(gin1[:], 0, B)], hn1, B, HS)
                do_ag(gin1, gout1)
                dma_back(gout1, hT1, 0, B)
                dma_back(gout1, hT1, B, 2 * B)

                # A-pair: layers 2,3 stacked (x = new h1 for BOTH)
                g0 = psg.tile([128, 288], f32, tag="g0", name="g0")
                g1 = psg.tile([128, 288], f32, tag="g1", name="g1")
                gate_mms(g0, g1, 128, "A",
                         x_chunks=[(hT1[:, c, :], "Ax", c) for c in range(NK)],
                         h_chunks=[(hTA[:, c, :], "Ah", c) for c in range(NK)])
                hnA = elementwise(g0, g1, 128, c_st["A"], (0, 128))
                gin2, gout2 = allgather(HS)
                gin3, gout3 = allgather(HS)
                transpose_to([(gin2[:], 0, B), (gin3[:], B, 2 * B)],
                             hnA, 128, HS)
                do_ag(gin2, gout2)
                do_ag(gin3, gout3)
                dma_back(gout2, hTA, 0, B)
                dma_back(gout3, hTA, B, 2 * B)

                # L4 (x = new h3)
                g0 = psg.tile([128, 288], f32, tag="g0", name="g0")
                g1 = psg.tile([128, 288], f32, tag="g1", name="g1")
                gate_mms(g0, g1, B, "L",
                         x_chunks=[(hTA[:, c, B:2 * B], "Lx", c) for c in range(NK)],
                         h_chunks=[(hTL[:, c, 0:B], "Lh", c) for c in range(NK)])
                hn4 = elementwise(g0, g1, B, c_st[4], (0, B))
                gin4, gout4 = allgather(HS)
                transpose_to([(gin4[:], 0, B)], hn4, B, HS)
                do_ag(gin4, gout4)
                dma_back(gout4, hTL, 0, B)

                # L5 (x = new h4)
                g0 = psg.tile([128, 288], f32, tag="g0", name="g0")
                g1 = psg.tile([128, 288], f32, tag="g1", name="g1")
                gate_mms(g0, g1, B, "L",
                         x_chunks=[(hTL[:, c, 0:B], "Lx", c) for c in range(NK)],
                         h_chunks=[(hTL[:, c, B:2 * B], "Lh", c) for c in range(NK)])
                hn5 = elementwise(g0, g1, B, c_st[5], (0, B))
                gin5, gout5 = allgather(HS)
                transpose_to([(gin5[:], 0, B)], hn5, B, HS)
                do_ag(gin5, gout5)
                dma_back(gout5, hTL, B, 2 * B)

                # heads (replicated on every core)
                ph = psh.tile([B, D_IN], f32, tag="ph", name="ph")
                heads = [(hTA, 0, B, 0, 12),
                         (hTA, B, 2 * B, 12, 24),
                         (hT1, 0, B, 24, 36),
                         (hTL, 0, B, 36, 45),
                         (hTL, B, 2 * B, 45, 54)]
                for src, lo, hi, olo, ohi in heads:
                    for c in range(NK):
                        nc.tensor.matmul(ph[:, olo:ohi],
                                         src[:, c, lo:hi].bitcast(f32),
                                         wh_all[:, c, olo:ohi],
                                         start=(c == 0), stop=False)
                    nc.tensor.matmul(ph[:, olo:ohi], ones[0:1, 0:B],
                                     hb1[0:1, olo:ohi],
                                     start=False, stop=True)
                pre = wk.tile([B, D_IN], f32, tag="pre", name="pre")
                nc.vector.tensor_tensor(pre[:], ph[:], x0b[:], op=OP.add)
                pre_bf = wk.tile([B, D_IN], bf16, tag="prebf", name="prebf")
                nc.vector.tensor_copy(pre_bf[:], pre[:])
                nc.sync.dma_start(out_d[:, t, :], pre_bf[:])
                if t < T_OUT - 1:
                    nc.vector.tensor_copy(x0b[:], pre[:])
                    ptq = pst.tile([128, 128], f32, tag="pt", name="pt")
                    nc.tensor.transpose(ptq[0:D_IN, 0:B], pre[0:B, 0:D_IN],
                                        ident[0:B, 0:B])
                    nc.scalar.copy(x0T[:], ptq[0:D_IN, 0:B])

    nc.compile()
    return nc


def _quant(w):
    s = float(np.abs(w).max())
    if s == 0.0:
        s = 1.0
    q = np.clip(np.rint(w * (127.0 / s)), -127, 127).astype(np.int8)
    return q, np.float32(s / 127.0)


NF = NK * GS
NH = NF // 2
NL = NF // 4


def _prep_inputs(inputs):
    scales = np.zeros(24, np.float32)

    # seven big weight tensors -> int6/int5 bit-planes per core
    wqh = np.empty((NC_, 7, 128, NH), np.uint8)
    wq6l = np.empty((NC_, N6, 128, NL), np.uint8)
    wq5l = np.empty((NC_, N5, 128, _N8), np.uint8)
    i6 = i5 = 0
    for i, (tag, key) in enumerate(TAGS):
        bits = TBITS[i]
        W = inputs[key]
        s = float(np.abs(W).max())
        if s == 0.0:
            s = 1.0
        lev = 31 if bits == 6 else 15
        sq = s / lev
        hmul = 4 if bits == 6 else 2
        scales[3 * i] = hmul * sq
        scales[3 * i + 1] = -(lev + 1) * sq  # -32*s6 / -16*s5
        scales[3 * i + 2] = sq
        v = (np.clip(np.rint(W * (1.0 / sq)), -lev, lev) + lev + 1).astype(np.uint8)
        # [4g, NC_, HS, NK, 128] -> per-core flat [NC_, 128(c), NK*GS]
        v5 = v.reshape(4, NC_, HS, NK, 128)[PERM]
        flat = v5.transpose(1, 4, 3, 0, 2).reshape(NC_, 128, NF)
        if bits == 6:
            hpl = flat >> 2
            lpl = flat & 3
            wqh[:, i] = hpl[:, :, 0:NH] | (hpl[:, :, NH:] << 4)
            wq6l[:, i6] = (lpl[:, :, 0:NL] | (lpl[:, :, NL:2 * NL] << 2)
                           | (lpl[:, :, 2 * NL:3 * NL] << 4)
                           | (lpl[:, :, 3 * NL:] << 6))
            i6 += 1
        else:
            hpl = flat >> 1
            lpl = flat & 1
            wqh[:, i] = hpl[:, :, 0:NH] | (hpl[:, :, NH:] << 4)
            acc = np.zeros((NC_, 128, _N8), np.uint8)
            for j in range(8):
                acc |= (lpl[:, :, j * _N8:(j + 1) * _N8] << j).astype(np.uint8)
            wq5l[:, i5] = acc
            i5 += 1

    q, s = _quant(inputs["Wih0"])
    scales[21] = s
    w0x8 = q.reshape(4, NC_, HS, 54)[PERM].transpose(1, 3, 0, 2) \
            .reshape(NC_, 54, GS)

    whcat = np.concatenate([inputs["W_leg1"], inputs["W_leg2"],
                            inputs["W_spine"], inputs["W_arm1"],
                            inputs["W_arm2"]], axis=1).astype(np.float32)
    qh, sh = _quant(whcat)
    scales[22] = sh
    wh8 = np.ascontiguousarray(qh.reshape(NK, 128, 54))
    wscale = np.broadcast_to(scales, (128, 24)).copy()

    # biases per tag, per-core gate-col order
    b4 = np.stack([(inputs["bih" + t] + inputs["bhh" + t]).astype(np.float32)
                   for t in "01AL"])                       # [4tag, 4608]
    b4 = b4.reshape(4, 4, NC_, HS)[:, PERM]                # [tag, g', core, HS]
    b_all = np.ascontiguousarray(b4.transpose(2, 0, 1, 3).reshape(NC_, 4, GS))

    hbias = np.concatenate([inputs["b_leg1"], inputs["b_leg2"],
                            inputs["b_spine"], inputs["b_arm1"],
                            inputs["b_arm2"]]).astype(np.float32)[None, :]

    # host-side encoder means
    hs_sum = inputs["hidden_states"].sum(axis=1, dtype=np.float64)
    cin = (inputs["cell_states"].mean(axis=1, dtype=np.float64)).astype(np.float32)
    h0m = (hs_sum / T_ENC).astype(np.float32)
    h1m = ((hs_sum + inputs["global_t_state"]) / (T_ENC + 1)).astype(np.float32)
    h0T = np.ascontiguousarray(h0m.T)          # [H, B]
    h1T = np.ascontiguousarray(h1m.T)

    p0 = np.ascontiguousarray(inputs["p"][:, 0, :]).astype(np.float32)

    in_maps = []
    for j in range(NC_):
        sl = slice(j * HS, (j + 1) * HS)
        sec = {
            "wscale": wscale,
            "bias": b_all[j],
            "hb1": hbias,
            "h0T": h0T[sl],
            "h1T": h1T[sl],
            "cin": cin[:, sl],
            "p0": p0,
            "wh8": wh8,
            "w0x8": w0x8[j],
            "wqh": wqh[j],
            "wq6l": wq6l[j],
            "wq5l": wq5l[j],
        }
        bl = np.empty(BLOB_BYTES, np.uint8)
        for name, arr in sec.items():
            off, nb = BLOB_OFF[name]
            flat = np.ascontiguousarray(arr).view(np.uint8).reshape(-1)
            assert flat.nbytes == nb, (name, flat.nbytes, nb)
            bl[off:off + nb] = flat
        in_maps.append({"blob": bl})
    return in_maps


def kernel(**inputs):
    global _compiled
    import concourse.bass_utils as bass_utils
    if _compiled is None:
        _compiled = _build()
    in_maps = _prep_inputs(inputs)
    res = bass_utils.run_bass_kernel_spmd(
        _compiled, in_maps, core_ids=list(range(NC_)))
    return np.asarray(res.results[0]["out"]).astype(np.float32)


# revision 5
# speedup vs baseline: 2.3281x; 2.3281x over previous
"""Kinematics LSTM decoder on 8 trn2 NeuronCores — wire-optimized.

The axon tunnel moves host->device bytes at ~25-45MB/s and device exec is
~6ms, so the per-call wall is dominated by input transfer plus a ~0.9s
per-call XLA+walrus recompile. Optimizations vs the 186MB/8.6s baseline:

- big LSTM weights shipped as packed int6 (Whh0/Wih1/Whh1) / int5
  (WihA/WhhA/WihL/WhhL) bit-planes, unpacked+dequantized on device into
  SBUF-resident f32r tiles (real-input l2rel 8.0e-3 vs the 2e-2 gate);
  Wih0 + output heads as int8
- encoder means (h0/h1/c_init) computed host-side; zeros/identity
  generated on device; head bias folded into the head-matmul PSUM group
- ALL per-core inputs packed into ONE uint8 blob (~3.6MB/core, 28.6MB
  total) so the tunnel sees a single large array; output in bf16
- jax persistent compilation cache enabled so warm calls skip the
  per-call NEFF recompile

Device strategy (unchanged from the original baseline): model-parallel
over the 4608 gate dim (576 gate cols / core = 144 h cols / core).
Recurrence: 25 steps x 6 cells; per-cell AllGather of the transposed h
slice through DRAM bounce buffers. Gates layout [batch, gatecols],
per-core col order [i f | o g]; matmuls in f32r. Layers 2,3 share
weights AND input -> batch-stacked (M=128).
"""
import numpy as np

B, T_ENC, D_IN, H, T_OUT = 64, 49, 54, 1152, 25
NC_ = 8          # cores
HS = H // NC_    # 144 h cols per core
GS = 4 * HS      # 576 gate cols per core
NK = H // 128    # 9 contraction chunks

PERM = [0, 1, 3, 2]  # pytorch gate order (i,f,g,o) -> per-core col order (i,f,o,g)
TAGS = [("0h", "Whh0"), ("1x", "Wih1"), ("1h", "Whh1"),
        ("Ax", "WihA"), ("Ah", "WhhA"), ("Lx", "WihL"), ("Lh", "WhhL")]
TAGIDX = {"0": 0, "1": 1, "A": 2, "L": 3}

_NF = NK * GS
_NH = _NF // 2
_NL = _NF // 4
_N8 = _NF // 8

# bits per big-weight tensor, in TAGS order (0h,1x,1h at 6; A/L cells at 5 —
# real-input sim: l2rel 7.9e-3, maxrel 9.7e-3 vs the 2e-2 gate)
TBITS = [6, 6, 6, 5, 5, 5, 5]
N6 = sum(1 for b in TBITS if b == 6)
N5 = sum(1 for b in TBITS if b == 5)


def _blob_layout():
    layout = [
        ("wscale", 128 * 24 * 4),
        ("bias", 4 * GS * 4),
        ("hb1", D_IN * 4),
        ("h0T", HS * B * 4),
        ("h1T", HS * B * 4),
        ("cin", B * HS * 4),
        ("p0", B * D_IN * 4),
        ("wh8", NK * 128 * 54),
        ("w0x8", 54 * GS),
        ("wqh", 7 * 128 * _NH),
        ("wq6l", N6 * 128 * _NL),
        ("wq5l", N5 * 128 * _N8),
    ]
    off, d = 0, {}
    for name, nb in layout:
        d[name] = (off, nb)
        off += nb
    return d, off


BLOB_OFF, BLOB_BYTES = _blob_layout()

_compiled = None


def _enable_jax_cache():
    """Persistent XLA executable cache: without it every
    run_bass_kernel_spmd call re-lowers + re-runs the walrus NEFF
    compile (~0.9s/call)."""
    try:
        import jax
        jax.config.update("jax_compilation_cache_dir", "/tmp/bass_jax_cache")
        jax.config.update("jax_persistent_cache_min_entry_size_bytes", -1)
        jax.config.update("jax_persistent_cache_min_compile_time_secs", 0)
    except Exception:
        pass


_enable_jax_cache()



def _build():
    import concourse.bass as bass
    import concourse.bacc as bacc
    import concourse.tile as tile
    import concourse.mybir as mybir

    f32 = mybir.dt.float32
    f32r = mybir.dt.float32r
    bf16 = mybir.dt.bfloat16
    i8 = mybir.dt.int8
    u8 = mybir.dt.uint8
    AF = mybir.ActivationFunctionType
    OP = mybir.AluOpType

    NF = NK * GS          # 5184 flat weight cols per partition
    NH = NF // 2          # 2592
    NL = NF // 4          # 1296

    nc = bacc.Bacc("TRN2", target_bir_lowering=False, debug=False,
                   num_devices=NC_)

    # single per-core input blob; section offsets must match _prep_inputs
    blob = nc.dram_tensor("blob", [BLOB_BYTES], u8, kind="ExternalInput")

    def bsec(name, dt_):
        off, nbytes = BLOB_OFF[name]
        ap = blob[off:off + nbytes]
        return ap if dt_ == u8 else ap.bitcast(dt_)

    out_d = nc.dram_tensor("out", [B, T_OUT, D_IN], bf16, kind="ExternalOutput")

    RG = [list(range(NC_))]

    with tile.TileContext(nc) as tc:
        with tc.tile_pool(name="wpool", bufs=1) as wp, \
             tc.tile_pool(name="stg", bufs=1) as stg, \
             tc.tile_pool(name="state", bufs=1) as st, \
             tc.tile_pool(name="work", bufs=3) as wk, \
             tc.tile_pool(name="hnewp", bufs=2) as hp, \
             tc.tile_pool(name="psg", bufs=2, space="PSUM") as psg, \
             tc.tile_pool(name="pst", bufs=2, space="PSUM") as pst, \
             tc.tile_pool(name="psh", bufs=1, space="PSUM") as psh, \
             tc.tile_pool(name="dram", bufs=6, space="DRAM") as dp:

            # ---- scales / misc constants ----
            wsc = wp.tile([128, 24], f32, tag="wsc", name="wsc")
            nc.sync.dma_start(
                wsc[:], bsec("wscale", f32).rearrange("(p f) -> p f", p=128))
            ones = wp.tile([1, 128], f32, tag="ones", name="ones")
            nc.vector.memset(ones[:], 1.0)
            ones128 = wp.tile([128, 128], f32, tag="ones128", name="ones128")
            nc.vector.memset(ones128[:], 1.0)
            ident = wp.tile([128, 128], f32, tag="ident", name="ident")
            nc.gpsimd.affine_select(ident[:], ones128[:], pattern=[[-1, 128]],
                                    compare_op=OP.is_equal, fill=0.0,
                                    base=0, channel_multiplier=1)
            hb1 = wp.tile([1, D_IN], f32, tag="hb1", name="hb1")
            nc.sync.dma_start(
                hb1[:], bsec("hb1", f32).rearrange("(p f) -> p f", p=1))
            boff = BLOB_OFF["bias"][0]
            b_sb = {}
            for ti, tg in enumerate("01AL"):
                b_sb[tg] = wp.tile([1, GS], f32, tag="b" + tg, name="b" + tg)
                nc.sync.dma_start(
                    b_sb[tg][:],
                    blob[boff + ti * GS * 4:boff + (ti + 1) * GS * 4]
                    .bitcast(f32).rearrange("(p f) -> p f", p=1))

            # ---- unpack int6/int5 weights into SBUF-resident f32r tiles ----
            # int6: v = round(w/s)+32 in [1,63]; h=v>>2 (4b), l=v&3 (2b);
            #       w = h*(4s) - 32s + l*s
            # int5: v = round(w/s)+16 in [1,31]; h=v>>1 (4b), l=v&1 (1b);
            #       w = h*(2s) - 16s + l*s
            N8 = NF // 8
            w_sb = {}
            hoff = BLOB_OFF["wqh"][0]
            l6off = BLOB_OFF["wq6l"][0]
            l5off = BLOB_OFF["wq5l"][0]
            i6 = i5 = 0
            for i, (tag, _) in enumerate(TAGS):
                bits = TBITS[i]
                sh = stg.tile([128, NH], u8, tag="sth", name="sth" + tag)
                nc.sync.dma_start(
                    sh[:], blob[hoff + i * 128 * NH:hoff + (i + 1) * 128 * NH]
                    .rearrange("(p f) -> p f", p=128))
                if bits == 6:
                    lsz = NL
                    loff = l6off + i6 * 128 * NL
                    i6 += 1
                else:
                    lsz = N8
                    loff = l5off + i5 * 128 * N8
                    i5 += 1
                sl = stg.tile([128, lsz], u8, tag=f"stl{bits}", name="stl" + tag)
                nc.sync.dma_start(
                    sl[:], blob[loff:loff + 128 * lsz]
                    .rearrange("(p f) -> p f", p=128))
                w_sb[tag] = wp.tile([128, NK, GS], f32r, tag="w" + tag, name="w" + tag)
                wf = w_sb[tag][:].rearrange("p a b -> p (a b)")
                shi = wsc[:, 3 * i:3 * i + 1]
                soff = wsc[:, 3 * i + 1:3 * i + 2]
                s1 = wsc[:, 3 * i + 2:3 * i + 3]
                # hi plane (4 bits either way)
                hsh = 2 if bits == 6 else 1
                th = stg.tile([128, NH], u8, tag="th", name="th" + tag)
                nc.vector.tensor_scalar(th[:], sh[:], 15, None, op0=OP.bitwise_and)
                nc.vector.tensor_scalar(wf[:, 0:NH], th[:], shi, soff,
                                        op0=OP.mult, op1=OP.add)
                nc.vector.tensor_scalar(th[:], sh[:], 4, None,
                                        op0=OP.logical_shift_right)
                nc.vector.tensor_scalar(wf[:, NH:NF], th[:], shi, soff,
                                        op0=OP.mult, op1=OP.add)
                # lo plane: 8 subranges of N8 columns each
                for k in range(8):
                    if bits == 6:
                        qd, hh = k // 2, k % 2
                        lsrc = sl[:, hh * N8:(hh + 1) * N8]
                        shift, mask, last = 2 * qd, 3, (qd == 3)
                    else:
                        lsrc = sl[:]
                        shift, mask, last = k, 1, (k == 7)
                    dst = wf[:, k * N8:(k + 1) * N8] if bits == 5 else \
                        wf[:, (k // 2) * NL + (k % 2) * N8:
                           (k // 2) * NL + (k % 2 + 1) * N8]
                    tl = stg.tile([128, N8], u8, tag="tl", name="tl")
                    if shift == 0:
                        nc.vector.tensor_scalar(tl[:], lsrc, mask, None,
                                                op0=OP.bitwise_and)
                    elif last:
                        nc.vector.tensor_scalar(tl[:], lsrc, shift, None,
                                                op0=OP.logical_shift_right)
                    else:
                        nc.vector.tensor_scalar(tl[:], lsrc, shift, mask,
                                                op0=OP.logical_shift_right,
                                                op1=OP.bitwise_and)
                    tf = stg.tile([128, N8], f32, tag="tf", name="tf")
                    nc.vector.tensor_scalar(tf[:], tl[:], s1, None,
                                            op0=OP.mult)
                    nc.vector.tensor_tensor(dst, dst, tf[:], op=OP.add)

            sg0 = stg.tile([54, GS], i8, tag="stg0", name="stg0x")
            nc.sync.dma_start(
                sg0[:], bsec("w0x8", i8).rearrange("(p f) -> p f", p=54))
            w_sb["0x"] = wp.tile([54, GS], f32r, tag="w0x", name="w0x")
            nc.vector.tensor_scalar(w_sb["0x"][:], sg0[:], wsc[0:54, 21:22],
                                    None, op0=OP.mult)
            sgh = stg.tile([128, NK, 54], i8, tag="stgh", name="stgh")
            nc.sync.dma_start(
                sgh[:], bsec("wh8", i8).rearrange("(c k n) -> k c n",
                                                  c=NK, k=128, n=54))
            wh_all = wp.tile([128, NK, 54], f32, tag="whall", name="whall")
            nc.vector.tensor_scalar(wh_all[:], sgh[:], wsc[:, 22:23],
                                    None, op0=OP.mult)

            # ---- persistent state ----
            hT0 = st.tile([128, NK, B], f32r, tag="hT0", name="hT0")        # h0.T
            hT1 = st.tile([128, NK, 2 * B], f32r, tag="hT1", name="hT1")    # h1.T duplicated
            hTA = st.tile([128, NK, 2 * B], f32r, tag="hTA", name="hTA")    # h2.T | h3.T
            hTL = st.tile([128, NK, 2 * B], f32r, tag="hTL", name="hTL")    # h4.T | h5.T
            c_st = {0: st.tile([B, HS], f32, tag="c0", name="c0"),
                    1: st.tile([B, HS], f32, tag="c1", name="c1"),
                    "A": st.tile([2 * B, HS], f32, tag="cA", name="cA"),
                    4: st.tile([B, HS], f32, tag="c4", name="c4"),
                    5: st.tile([B, HS], f32, tag="c5", name="c5")}
            x0b = st.tile([B, D_IN], f32, tag="x0b", name="x0b")
            x0T = st.tile([D_IN, B], f32r, tag="x0T", name="x0T")

            zf = wp.tile([128, 2 * B], f32, tag="zf", name="zf")
            nc.vector.memset(zf[:], 0.0)
            for c in range(NK):
                nc.scalar.copy(hTA[:, c, :], zf[:])
                nc.scalar.copy(hTL[:, c, :], zf[:])
            nc.vector.memset(c_st["A"][:], 0.0)
            nc.vector.memset(c_st[4][:], 0.0)
            nc.vector.memset(c_st[5][:], 0.0)

            r32 = lambda ap: ap.bitcast(f32r)

            def transpose_to(dst_dram_slices, src_sb, rows, cols):
                """src_sb [rows<=128, cols] -> transposed [cols, rows] written
                to dram col-splits."""
                done = 0
                while done < cols:
                    n = min(128, cols - done)
                    pt = pst.tile([128, 128], f32, tag="pt", name="pt")
                    nc.tensor.transpose(pt[0:n, 0:rows],
                                        src_sb[0:rows, done:done + n],
                                        ident[0:rows, 0:rows])
                    cp = wk.tile([128, 128], f32r, tag="tcp", name="tcp")
                    nc.scalar.copy(cp[0:n, 0:rows], pt[0:n, 0:rows])
                    for (dap, lo, hi) in dst_dram_slices:
                        nc.sync.dma_start(dap[done:done + n, :],
                                          cp[0:n, lo:hi])
                    done += n

            def allgather(n_rows):
                gin = dp.tile([n_rows, B], f32r, tag="agin", name="agin")
                gout = dp.tile([NC_ * n_rows, B], f32r, tag="agout", name="agout")
                return gin, gout

            def do_ag(gin, gout):
                nc.gpsimd.collective_compute(
                    "AllGather", OP.bypass, replica_groups=RG,
                    ins=[gin[:].opt()], outs=[gout[:].opt()])

            def dma_back(gout, dst, lo, hi):
                nc.sync.dma_start(
                    dst[:, :, lo:hi],
                    gout[:].rearrange("(c k) n -> k c n", k=128))

            # ---- prologue: states from host-computed means ----
            nc.sync.dma_start(
                c_st[0][:], bsec("cin", f32).rearrange("(p f) -> p f", p=B))
            nc.vector.tensor_copy(c_st[1][:], c_st[0][:])

            for (nm, dsts) in (("h0T", [(hT0, 0, B)]),
                               ("h1T", [(hT1, 0, B), (hT1, B, 2 * B)])):
                gin, gout = allgather(HS)
                nc.sync.dma_start(
                    gin[:], bsec(nm, f32r).rearrange("(p f) -> p f", p=HS))
                do_ag(gin, gout)
                for (dst, lo, hi) in dsts:
                    dma_back(gout, dst, lo, hi)

            # x0
            nc.sync.dma_start(
                x0b[:], bsec("p0", f32).rearrange("(p f) -> p f", p=B))
            ptp = pst.tile([128, 128], f32, tag="pt", name="pt")
            nc.tensor.transpose(ptp[0:D_IN, 0:B], x0b[0:B, 0:D_IN],
                                ident[0:B, 0:B])
            nc.scalar.copy(x0T[:], ptp[0:D_IN, 0:B])

            # ---- helpers for the recurrence ----
            def gate_mms(g0, g1, rows, wtag, x_chunks, h_chunks):
                first = True
                for (lhsT, wkey, c) in h_chunks + x_chunks:
                    if wkey == "0x":
                        r0 = w_sb["0x"][0:54, 0:288]
                        r1 = w_sb["0x"][0:54, 288:GS]
                    else:
                        r0 = w_sb[wkey][:, c, 0:288]
                        r1 = w_sb[wkey][:, c, 288:GS]
                    nc.tensor.matmul(g0[0:rows, :], r32(lhsT), r32(r0),
                                     start=first, stop=False)
                    nc.tensor.matmul(g1[0:rows, :], r32(lhsT), r32(r1),
                                     start=first, stop=False)
                    first = False
                nc.tensor.matmul(g0[0:rows, :], ones[0:1, 0:rows],
                                 b_sb[wtag][0:1, 0:288],
                                 start=False, stop=True)
                nc.tensor.matmul(g1[0:rows, :], ones[0:1, 0:rows],
                                 b_sb[wtag][0:1, 288:GS],
                                 start=False, stop=True)

            def elementwise(g0, g1, rows, c_tile, crange):
                """gates [i f | o g]; returns h_new sbuf tile [rows, HS]"""
                sif = wk.tile([128, 2 * HS], f32, tag="sif", name="sif")
                nc.scalar.activation(sif[0:rows, :], g0[0:rows, :], AF.Sigmoid)
                so = wk.tile([128, HS], f32, tag="so", name="so")
                nc.scalar.activation(so[0:rows, :], g1[0:rows, 0:HS], AF.Sigmoid)
                tg = wk.tile([128, HS], f32, tag="tg", name="tg")
                nc.scalar.activation(tg[0:rows, :], g1[0:rows, HS:2 * HS], AF.Tanh)
                t1 = wk.tile([128, HS], f32, tag="t1", name="t1")
                cs = c_tile[crange[0]:crange[1], :]
                nc.vector.tensor_tensor(t1[0:rows, :], sif[0:rows, HS:2 * HS],
                                        cs, op=OP.mult)
                t2 = wk.tile([128, HS], f32, tag="t2", name="t2")
                nc.vector.tensor_tensor(t2[0:rows, :], sif[0:rows, 0:HS],
                                        tg[0:rows, :], op=OP.mult)
                nc.vector.tensor_tensor(cs, t1[0:rows, :], t2[0:rows, :],
                                        op=OP.add)
                tc_ = wk.tile([128, HS], f32, tag="tc", name="tc")
                nc.scalar.activation(tc_[0:rows, :], cs, AF.Tanh)
                hn = hp.tile([128, HS], f32, tag="hnew", name="hnew")
                nc.vector.tensor_tensor(hn[0:rows, :], so[0:rows, :],
                                        tc_[0:rows, :], op=OP.mult)
                return hn

            # ---- recurrence ----
            for t in range(T_OUT):
                # L0
                g0 = psg.tile([128, 288], f32, tag="g0", name="g0")
                g1 = psg.tile([128, 288], f32, tag="g1", name="g1")
                gate_mms(g0, g1, B, "0",
                         x_chunks=[(x0T[0:54, 0:B], "0x", 0)],
                         h_chunks=[(hT0[:, c, :], "0h", c) for c in range(NK)])
                hn0 = elementwise(g0, g1, B, c_st[0], (0, B))
                gin0, gout0 = allgather(HS)
                transpose_to([(gin0[:], 0, B)], hn0, B, HS)
                do_ag(gin0, gout0)
                dma_back(gout0, hT0, 0, B)

                # L1 (x = new h0)
                g0 = psg.tile([128, 288], f32, tag="g0", name="g0")
                g1 = psg.tile([128, 288], f32, tag="g1", name="g1")
                gate_mms(g0, g1, B, "1",
                         x_chunks=[(hT0[:, c, :], "1x", c) for c in range(NK)],
                         h_chunks=[(hT1[:, c, 0:B], "1h", c) for c in range(NK)])
                hn1 = elementwise(g0, g1, B, c_st[1], (0, B))
                gin1, gout1 = allgather(HS)
                transpose_to([(gin1[:], 0, B)], hn1, B, HS)
                do_ag(gin1, gout1)
                dma_back(gout1, hT1, 0, B)
                dma_back(gout1, hT1, B, 2 * B)

                # A-pair: layers 2,3 stacked (x = new h1 for BOTH)
                g0 = psg.tile([128, 288], f32, tag="g0", name="g0")
                g1 = psg.tile([128, 288], f32, tag="g1", name="g1")
                gate_mms(g0, g1, 128, "A",
                         x_chunks=[(hT1[:, c, :], "Ax", c) for c in range(NK)],
                         h_chunks=[(hTA[:, c, :], "Ah", c) for c in range(NK)])
                hnA = elementwise(g0, g1, 128, c_st["A"], (0, 128))
                gin2, gout2 = allgather(HS)
                gin3, gout3 = allgather(HS)
                transpose_to([(gin2[:], 0, B), (gin3[:], B, 2 * B)],
                             hnA, 128, HS)
                do_ag(gin2, gout2)
                do_ag(gin3, gout3)
                dma_back(gout2, hTA, 0, B)
                dma_back(gout3, hTA, B, 2 * B)

                # L4 (x = new h3)
                g0 = psg.tile([128, 288], f32, tag="g0", name="g0")
                g1 = psg.tile([128, 288], f32, tag="g1", name="g1")
                gate_mms(g0, g1, B, "L",
                         x_chunks=[(hTA[:, c, B:2 * B], "Lx", c) for c in range(NK)],
                         h_chunks=[(hTL[:, c, 0:B], "Lh", c) for c in range(NK)])
                hn4 = elementwise(g0, g1, B, c_st[4], (0, B))
                gin4, gout4 = allgather(HS)
                transpose_to([(gin4[:], 0, B)], hn4, B, HS)
                do_ag(gin4, gout4)
                dma_back(gout4, hTL, 0, B)

                # L5 (x = new h4)
                g0 = psg.tile([128, 288], f32, tag="g0", name="g0")
                g1 = psg.tile([128, 288], f32, tag="g1", name="g1")
                gate_mms(g0, g1, B, "L",
                         x_chunks=[(hTL[:, c, 0:B], "Lx", c) for c in range(NK)],
                         h_chunks=[(hTL[:, c, B:2 * B], "Lh", c) for c in range(NK)])
                hn5 = elementwise(g0, g1, B, c_st[5], (0, B))
                gin5, gout5 = allgather(HS)
                transpose_to([(gin5[:], 0, B)], hn5, B, HS)
                do_ag(gin5, gout5)
                dma_back(gout5, hTL, B, 2 * B)

                # heads (replicated on every core)
                ph = psh.tile([B, D_IN], f32, tag="ph", name="ph")
                heads = [(hTA, 0, B, 0, 12),
                         (hTA, B, 2 * B, 12, 24),
                         (hT1, 0, B, 24, 36),
                         (hTL, 0, B, 36, 45),
                         (hTL, B, 2 * B, 45, 54)]
                for src, lo, hi, olo, ohi in heads:
                    for c in range(NK):
                        nc.tensor.matmul(ph[:, olo:ohi],
                                         src[:, c, lo:hi].bitcast(f32),
                                         wh_all[:, c, olo:ohi],
                                         start=(c == 0), stop=False)
                    nc.tensor.matmul(ph[:, olo:ohi], ones[0:1, 0:B],
                                     hb1[0:1, olo:ohi],
                                     start=False, stop=True)
                pre = wk.tile([B, D_IN], f32, tag="pre", name="pre")
                nc.vector.tensor_tensor(pre[:], ph[:], x0b[:], op=OP.add)
                pre_bf = wk.tile([B, D_IN], bf16, tag="prebf", name="prebf")
                nc.vector.tensor_copy(pre_bf[:], pre[:])
                nc.sync.dma_start(out_d[:, t, :], pre_bf[:])
                if t < T_OUT - 1:
                    nc.vector.tensor_copy(x0b[:], pre[:])
                    ptq = pst.tile([128, 128], f32, tag="pt", name="pt")
                    nc.tensor.transpose(ptq[0:D_IN, 0:B], pre[0:B, 0:D_IN],
                                        ident[0:B, 0:B])
                    nc.scalar.copy(x0T[:], ptq[0:D_IN, 0:B])

    nc.compile()
    return nc


def _quant(w):
    s = float(np.abs(w).max())
    if s == 0.0:
        s = 1.0
    q = np.clip(np.rint(w * (127.0 / s)), -127, 127).astype(np.int8)
    return q, np.float32(s / 127.0)


NF = NK * GS
NH = NF // 2
NL = NF // 4


def _prep_inputs(inputs):
    scales = np.zeros(24, np.float32)

    # seven big weight tensors -> int6/int5 bit-planes per core
    wqh = np.empty((NC_, 7, 128, NH), np.uint8)
    wq6l = np.empty((NC_, N6, 128, NL), np.uint8)
    wq5l = np.empty((NC_, N5, 128, _N8), np.uint8)
    i6 = i5 = 0
    for i, (tag, key) in enumerate(TAGS):
        bits = TBITS[i]
        W = inputs[key]
        s = float(np.abs(W).max())
        if s == 0.0:
            s = 1.0
        lev = 31 if bits == 6 else 15
        sq = s / lev
        hmul = 4 if bits == 6 else 2
        scales[3 * i] = hmul * sq
        scales[3 * i + 1] = -(lev + 1) * sq  # -32*s6 / -16*s5
        scales[3 * i + 2] = sq
        v = (np.clip(np.rint(W * (1.0 / sq)), -lev, lev) + lev + 1).astype(np.uint8)
        # [4g, NC_, HS, NK, 128] -> per-core flat [NC_, 128(c), NK*GS]
        v5 = v.reshape(4, NC_, HS, NK, 128)[PERM]
        flat = v5.transpose(1, 4, 3, 0, 2).reshape(NC_, 128, NF)
        if bits == 6:
            hpl = flat >> 2
            lpl = flat & 3
            wqh[:, i] = hpl[:, :, 0:NH] | (hpl[:, :, NH:] << 4)
            wq6l[:, i6] = (lpl[:, :, 0:NL] | (lpl[:, :, NL:2 * NL] << 2)
                           | (lpl[:, :, 2 * NL:3 * NL] << 4)
                           | (lpl[:, :, 3 * NL:] << 6))
            i6 += 1
        else:
            hpl = flat >> 1
            lpl = flat & 1
            wqh[:, i] = hpl[:, :, 0:NH] | (hpl[:, :, NH:] << 4)
            acc = np.zeros((NC_, 128, _N8), np.uint8)
            for j in range(8):
                acc |= (lpl[:, :, j * _N8:(j + 1) * _N8] << j).astype(np.uint8)
            wq5l[:, i5] = acc
            i5 += 1

    q, s = _quant(inputs["Wih0"])
    scales[21] = s
    w0x8 = q.reshape(4, NC_, HS, 54)[PERM].transpose(1, 3, 0, 2) \
            .reshape(NC_, 54, GS)

    whcat = np.concatenate([inputs["W_leg1"], inputs["W_leg2"],
                            inputs["W_spine"], inputs["W_arm1"],
                            inputs["W_arm2"]], axis=1).astype(np.float32)
    qh, sh = _quant(whcat)
    scales[22] = sh
    wh8 = np.ascontiguousarray(qh.reshape(NK, 128, 54))
    wscale = np.broadcast_to(scales, (128, 24)).copy()

    # biases per tag, per-core gate-col order
    b4 = np.stack([(inputs["bih" + t] + inputs["bhh" + t]).astype(np.float32)
                   for t in "01AL"])                       # [4tag, 4608]
    b4 = b4.reshape(4, 4, NC_, HS)[:, PERM]                # [tag, g', core, HS]
    b_all = np.ascontiguousarray(b4.transpose(2, 0, 1, 3).reshape(NC_, 4, GS))

    hbias = np.concatenate([inputs["b_leg1"], inputs["b_leg2"],
                            inputs["b_spine"], inputs["b_arm1"],
                            inputs["b_arm2"]]).astype(np.float32)[None, :]

    # host-side encoder means
    hs_sum = inputs["hidden_states"].sum(axis=1, dtype=np.float64)
    cin = (inputs["cell_states"].mean(axis=1, dtype=np.float64)).astype(np.float32)
    h0m = (hs_sum / T_ENC).astype(np.float32)
    h1m = ((hs_sum + inputs["global_t_state"]) / (T_ENC + 1)).astype(np.float32)
    h0T = np.ascontiguousarray(h0m.T)          # [H, B]
    h1T = np.ascontiguousarray(h1m.T)

    p0 = np.ascontiguousarray(inputs["p"][:, 0, :]).astype(np.float32)

    in_maps = []
    for j in range(NC_):
        sl = slice(j * HS, (j + 1) * HS)
        sec = {
            "wscale": wscale,
            "bias": b_all[j],
            "hb1": hbias,
            "h0T": h0T[sl],
            "h1T": h1T[sl],
            "cin": cin[:, sl],
            "p0": p0,
            "wh8": wh8,
            "w0x8": w0x8[j],
            "wqh": wqh[j],
            "wq6l": wq6l[j],
            "wq5l": wq5l[j],
        }
        bl = np.empty(BLOB_BYTES, np.uint8)
        for name, arr in sec.items():
            off, nb = BLOB_OFF[name]
            flat = np.ascontiguousarray(arr).view(np.uint8).reshape(-1)
            assert flat.nbytes == nb, (name, flat.nbytes, nb)
            bl[off:off + nb] = flat
        in_maps.append({"blob": bl})
    return in_maps


def kernel(**inputs):
    global _compiled
    import concourse.bass_utils as bass_utils
    if _compiled is None:
        _compiled = _build()
    in_maps = _prep_inputs(inputs)
    res = bass_utils.run_bass_kernel_spmd(
        _compiled, in_maps, core_ids=list(range(NC_)))
    return np.asarray(res.results[0]["out"]).astype(np.float32)


# revision 6
# speedup vs baseline: 2.5754x; 1.1062x over previous
"""Kinematics LSTM decoder on 8 trn2 NeuronCores — wire-optimized.

The axon tunnel moves host->device bytes at ~25-45MB/s and device exec is
~6ms, so the per-call wall is dominated by input transfer plus a ~0.9s
per-call XLA+walrus recompile. Optimizations vs the 186MB/8.6s baseline:

- big LSTM weights shipped as packed int6 (Whh0/Wih1/Whh1) / int5
  (WihA/WhhA/WihL/WhhL) bit-planes, unpacked+dequantized on device into
  SBUF-resident f32r tiles (real-input l2rel 8.0e-3 vs the 2e-2 gate);
  Wih0 + output heads as int8
- encoder means (h0/h1/c_init) computed host-side; zeros/identity
  generated on device; head bias folded into the head-matmul PSUM group
- ALL per-core inputs packed into ONE uint8 blob (~3.6MB/core, 28.6MB
  total) so the tunnel sees a single large array; output in bf16
- jax persistent compilation cache enabled so warm calls skip the
  per-call NEFF recompile

Device strategy (unchanged from the original baseline): model-parallel
over the 4608 gate dim (576 gate cols / core = 144 h cols / core).
Recurrence: 25 steps x 6 cells; per-cell AllGather of the transposed h
slice through DRAM bounce buffers. Gates layout [batch, gatecols],
per-core col order [i f | o g]; matmuls in f32r. Layers 2,3 share
weights AND input -> batch-stacked (M=128).
"""
import numpy as np

B, T_ENC, D_IN, H, T_OUT = 64, 49, 54, 1152, 25
NC_ = 8          # cores
HS = H // NC_    # 144 h cols per core
GS = 4 * HS      # 576 gate cols per core
NK = H // 128    # 9 contraction chunks

PERM = [0, 1, 3, 2]  # pytorch gate order (i,f,g,o) -> per-core col order (i,f,o,g)
TAGS = [("0h", "Whh0"), ("1x", "Wih1"), ("1h", "Whh1"),
        ("Ax", "WihA"), ("Ah", "WhhA"), ("Lx", "WihL"), ("Lh", "WhhL")]
TAGIDX = {"0": 0, "1": 1, "A": 2, "L": 3}

_NF = NK * GS
_NH = _NF // 2
_NL = _NF // 4
_N8 = _NF // 8

# bits per big-weight tensor, in TAGS order (0h,1x,1h at 6; A/L cells at 5 —
# real-input sim: l2rel 7.9e-3, maxrel 9.7e-3 vs the 2e-2 gate)
TBITS = [6, 6, 6, 5, 5, 5, 5]
N6 = sum(1 for b in TBITS if b == 6)
N5 = sum(1 for b in TBITS if b == 5)


def _blob_layout():
    layout = [
        ("wscale", 128 * 24 * 4),
        ("bias", 4 * GS * 4),
        ("hb1", D_IN * 4),
        ("h0T", HS * B * 4),
        ("h1T", HS * B * 4),
        ("cin", B * HS * 4),
        ("p0", B * D_IN * 4),
        ("wh8", NK * 128 * 54),
        ("w0x8", 54 * GS),
        ("wqh", 7 * 128 * _NH),
        ("wq6l", N6 * 128 * _NL),
        ("wq5l", N5 * 128 * _N8),
    ]
    off, d = 0, {}
    for name, nb in layout:
        d[name] = (off, nb)
        off += nb
    return d, off


BLOB_OFF, BLOB_BYTES = _blob_layout()

_compiled = None


def _enable_jax_cache():
    """Persistent XLA executable cache: without it every
    run_bass_kernel_spmd call re-lowers + re-runs the walrus NEFF
    compile (~0.9s/call)."""
    try:
        import jax
        jax.config.update("jax_compilation_cache_dir", "/tmp/bass_jax_cache")
        jax.config.update("jax_persistent_cache_min_entry_size_bytes", -1)
        jax.config.update("jax_persistent_cache_min_compile_time_secs", 0)
    except Exception:
        pass


_enable_jax_cache()



def _build():
    import concourse.bass as bass
    import concourse.bacc as bacc
    import concourse.tile as tile
    import concourse.mybir as mybir

    f32 = mybir.dt.float32
    f32r = mybir.dt.float32r
    bf16 = mybir.dt.bfloat16
    i8 = mybir.dt.int8
    u8 = mybir.dt.uint8
    AF = mybir.ActivationFunctionType
    OP = mybir.AluOpType

    NF = NK * GS          # 5184 flat weight cols per partition
    NH = NF // 2          # 2592
    NL = NF // 4          # 1296

    nc = bacc.Bacc("TRN2", target_bir_lowering=False, debug=False,
                   num_devices=NC_)

    # single per-core input blob; section offsets must match _prep_inputs
    blob = nc.dram_tensor("blob", [BLOB_BYTES], u8, kind="ExternalInput")

    def bsec(name, dt_):
        off, nbytes = BLOB_OFF[name]
        ap = blob[off:off + nbytes]
        return ap if dt_ == u8 else ap.bitcast(dt_)

    # Every core computes bit-identical outputs; ReduceScatter of pre/8
    # leaves each core exactly its 1/8 shard, cutting the donated-zeros
    # upload and the 8-core result fetch from 1.38MB to 173KB each way.
    NOUT = T_OUT * B * D_IN          # 86400
    NSH = NOUT // NC_                # 10800 elements per core
    out_d = nc.dram_tensor("out", [NSH], bf16, kind="ExternalOutput")

    RG = [list(range(NC_))]

    with tile.TileContext(nc) as tc:
        with tc.tile_pool(name="wpool", bufs=1) as wp, \
             tc.tile_pool(name="stg", bufs=1) as stg, \
             tc.tile_pool(name="state", bufs=1) as st, \
             tc.tile_pool(name="work", bufs=3) as wk, \
             tc.tile_pool(name="hnewp", bufs=2) as hp, \
             tc.tile_pool(name="psg", bufs=2, space="PSUM") as psg, \
             tc.tile_pool(name="pst", bufs=2, space="PSUM") as pst, \
             tc.tile_pool(name="psh", bufs=1, space="PSUM") as psh, \
             tc.tile_pool(name="dram", bufs=6, space="DRAM") as dp:

            # ---- scales / misc constants ----
            wsc = wp.tile([128, 24], f32, tag="wsc", name="wsc")
            nc.sync.dma_start(
                wsc[:], bsec("wscale", f32).rearrange("(p f) -> p f", p=128))
            ones = wp.tile([1, 128], f32, tag="ones", name="ones")
            nc.vector.memset(ones[:], 1.0)
            ones128 = wp.tile([128, 128], f32, tag="ones128", name="ones128")
            nc.vector.memset(ones128[:], 1.0)
            ident = wp.tile([128, 128], f32, tag="ident", name="ident")
            nc.gpsimd.affine_select(ident[:], ones128[:], pattern=[[-1, 128]],
                                    compare_op=OP.is_equal, fill=0.0,
                                    base=0, channel_multiplier=1)
            hb1 = wp.tile([1, D_IN], f32, tag="hb1", name="hb1")
            nc.sync.dma_start(
                hb1[:], bsec("hb1", f32).rearrange("(p f) -> p f", p=1))
            boff = BLOB_OFF["bias"][0]
            b_sb = {}
            for ti, tg in enumerate("01AL"):
                b_sb[tg] = wp.tile([1, GS], f32, tag="b" + tg, name="b" + tg)
                nc.sync.dma_start(
                    b_sb[tg][:],
                    blob[boff + ti * GS * 4:boff + (ti + 1) * GS * 4]
                    .bitcast(f32).rearrange("(p f) -> p f", p=1))

            # ---- unpack int6/int5 weights into SBUF-resident f32r tiles ----
            # int6: v = round(w/s)+32 in [1,63]; h=v>>2 (4b), l=v&3 (2b);
            #       w = h*(4s) - 32s + l*s
            # int5: v = round(w/s)+16 in [1,31]; h=v>>1 (4b), l=v&1 (1b);
            #       w = h*(2s) - 16s + l*s
            N8 = NF // 8
            w_sb = {}
            hoff = BLOB_OFF["wqh"][0]
            l6off = BLOB_OFF["wq6l"][0]
            l5off = BLOB_OFF["wq5l"][0]
            i6 = i5 = 0
            for i, (tag, _) in enumerate(TAGS):
                bits = TBITS[i]
                sh = stg.tile([128, NH], u8, tag="sth", name="sth" + tag)
                nc.sync.dma_start(
                    sh[:], blob[hoff + i * 128 * NH:hoff + (i + 1) * 128 * NH]
                    .rearrange("(p f) -> p f", p=128))
                if bits == 6:
                    lsz = NL
                    loff = l6off + i6 * 128 * NL
                    i6 += 1
                else:
                    lsz = N8
                    loff = l5off + i5 * 128 * N8
                    i5 += 1
                sl = stg.tile([128, lsz], u8, tag=f"stl{bits}", name="stl" + tag)
                nc.sync.dma_start(
                    sl[:], blob[loff:loff + 128 * lsz]
                    .rearrange("(p f) -> p f", p=128))
                w_sb[tag] = wp.tile([128, NK, GS], f32r, tag="w" + tag, name="w" + tag)
                wf = w_sb[tag][:].rearrange("p a b -> p (a b)")
                shi = wsc[:, 3 * i:3 * i + 1]
                soff = wsc[:, 3 * i + 1:3 * i + 2]
                s1 = wsc[:, 3 * i + 2:3 * i + 3]
                # hi plane (4 bits either way)
                hsh = 2 if bits == 6 else 1
                th = stg.tile([128, NH], u8, tag="th", name="th" + tag)
                nc.vector.tensor_scalar(th[:], sh[:], 15, None, op0=OP.bitwise_and)
                nc.vector.tensor_scalar(wf[:, 0:NH], th[:], shi, soff,
                                        op0=OP.mult, op1=OP.add)
                nc.vector.tensor_scalar(th[:], sh[:], 4, None,
                                        op0=OP.logical_shift_right)
                nc.vector.tensor_scalar(wf[:, NH:NF], th[:], shi, soff,
                                        op0=OP.mult, op1=OP.add)
                # lo plane: 8 subranges of N8 columns each
                for k in range(8):
                    if bits == 6:
                        qd, hh = k // 2, k % 2
                        lsrc = sl[:, hh * N8:(hh + 1) * N8]
                        shift, mask, last = 2 * qd, 3, (qd == 3)
                    else:
                        lsrc = sl[:]
                        shift, mask, last = k, 1, (k == 7)
                    dst = wf[:, k * N8:(k + 1) * N8] if bits == 5 else \
                        wf[:, (k // 2) * NL + (k % 2) * N8:
                           (k // 2) * NL + (k % 2 + 1) * N8]
                    tl = stg.tile([128, N8], u8, tag="tl", name="tl")
                    if shift == 0:
                        nc.vector.tensor_scalar(tl[:], lsrc, mask, None,
                                                op0=OP.bitwise_and)
                    elif last:
                        nc.vector.tensor_scalar(tl[:], lsrc, shift, None,
                                                op0=OP.logical_shift_right)
                    else:
                        nc.vector.tensor_scalar(tl[:], lsrc, shift, mask,
                                                op0=OP.logical_shift_right,
                                                op1=OP.bitwise_and)
                    tf = stg.tile([128, N8], f32, tag="tf", name="tf")
                    nc.vector.tensor_scalar(tf[:], tl[:], s1, None,
                                            op0=OP.mult)
                    nc.vector.tensor_tensor(dst, dst, tf[:], op=OP.add)

            sg0 = stg.tile([54, GS], i8, tag="stg0", name="stg0x")
            nc.sync.dma_start(
                sg0[:], bsec("w0x8", i8).rearrange("(p f) -> p f", p=54))
            w_sb["0x"] = wp.tile([54, GS], f32r, tag="w0x", name="w0x")
            nc.vector.tensor_scalar(w_sb["0x"][:], sg0[:], wsc[0:54, 21:22],
                                    None, op0=OP.mult)
            sgh = stg.tile([128, NK, 54], i8, tag="stgh", name="stgh")
            nc.sync.dma_start(
                sgh[:], bsec("wh8", i8).rearrange("(c k n) -> k c n",
                                                  c=NK, k=128, n=54))
            wh_all = wp.tile([128, NK, 54], f32, tag="whall", name="whall")
            nc.vector.tensor_scalar(wh_all[:], sgh[:], wsc[:, 22:23],
                                    None, op0=OP.mult)

            # ---- persistent state ----
            hT0 = st.tile([128, NK, B], f32r, tag="hT0", name="hT0")        # h0.T
            hT1 = st.tile([128, NK, 2 * B], f32r, tag="hT1", name="hT1")    # h1.T duplicated
            hTA = st.tile([128, NK, 2 * B], f32r, tag="hTA", name="hTA")    # h2.T | h3.T
            hTL = st.tile([128, NK, 2 * B], f32r, tag="hTL", name="hTL")    # h4.T | h5.T
            c_st = {0: st.tile([B, HS], f32, tag="c0", name="c0"),
                    1: st.tile([B, HS], f32, tag="c1", name="c1"),
                    "A": st.tile([2 * B, HS], f32, tag="cA", name="cA"),
                    4: st.tile([B, HS], f32, tag="c4", name="c4"),
                    5: st.tile([B, HS], f32, tag="c5", name="c5")}
            x0b = st.tile([B, D_IN], f32, tag="x0b", name="x0b")
            x0T = st.tile([D_IN, B], f32r, tag="x0T", name="x0T")

            zf = wp.tile([128, 2 * B], f32, tag="zf", name="zf")
            nc.vector.memset(zf[:], 0.0)
            for c in range(NK):
                nc.scalar.copy(hTA[:, c, :], zf[:])
                nc.scalar.copy(hTL[:, c, :], zf[:])
            nc.vector.memset(c_st["A"][:], 0.0)
            nc.vector.memset(c_st[4][:], 0.0)
            nc.vector.memset(c_st[5][:], 0.0)

            r32 = lambda ap: ap.bitcast(f32r)

            def transpose_to(dst_dram_slices, src_sb, rows, cols):
                """src_sb [rows<=128, cols] -> transposed [cols, rows] written
                to dram col-splits."""
                done = 0
                while done < cols:
                    n = min(128, cols - done)
                    pt = pst.tile([128, 128], f32, tag="pt", name="pt")
                    nc.tensor.transpose(pt[0:n, 0:rows],
                                        src_sb[0:rows, done:done + n],
                                        ident[0:rows, 0:rows])
                    cp = wk.tile([128, 128], f32r, tag="tcp", name="tcp")
                    nc.scalar.copy(cp[0:n, 0:rows], pt[0:n, 0:rows])
                    for (dap, lo, hi) in dst_dram_slices:
                        nc.sync.dma_start(dap[done:done + n, :],
                                          cp[0:n, lo:hi])
                    done += n

            def allgather(n_rows):
                gin = dp.tile([n_rows, B], f32r, tag="agin", name="agin")
                gout = dp.tile([NC_ * n_rows, B], f32r, tag="agout", name="agout")
                return gin, gout

            def do_ag(gin, gout):
                nc.gpsimd.collective_compute(
                    "AllGather", OP.bypass, replica_groups=RG,
                    ins=[gin[:].opt()], outs=[gout[:].opt()])

            def dma_back(gout, dst, lo, hi):
                nc.sync.dma_start(
                    dst[:, :, lo:hi],
                    gout[:].rearrange("(c k) n -> k c n", k=128))

            # ---- prologue: states from host-computed means ----
            nc.sync.dma_start(
                c_st[0][:], bsec("cin", f32).rearrange("(p f) -> p f", p=B))
            nc.vector.tensor_copy(c_st[1][:], c_st[0][:])

            for (nm, dsts) in (("h0T", [(hT0, 0, B)]),
                               ("h1T", [(hT1, 0, B), (hT1, B, 2 * B)])):
                gin, gout = allgather(HS)
                nc.sync.dma_start(
                    gin[:], bsec(nm, f32r).rearrange("(p f) -> p f", p=HS))
                do_ag(gin, gout)
                for (dst, lo, hi) in dsts:
                    dma_back(gout, dst, lo, hi)

            # x0
            nc.sync.dma_start(
                x0b[:], bsec("p0", f32).rearrange("(p f) -> p f", p=B))

            # staging buffer for all 25 steps' outputs (f32, pre-scaled by
            # 1/8 so the final ReduceScatter(add) over 8 identical copies
            # reconstructs the value)
            outfull = dp.tile([T_OUT, B, D_IN], f32, tag="outfull",
                              name="outfull")
            ptp = pst.tile([128, 128], f32, tag="pt", name="pt")
            nc.tensor.transpose(ptp[0:D_IN, 0:B], x0b[0:B, 0:D_IN],
                                ident[0:B, 0:B])
            nc.scalar.copy(x0T[:], ptp[0:D_IN, 0:B])

            # ---- helpers for the recurrence ----
            def gate_mms(g0, g1, rows, wtag, x_chunks, h_chunks):
                first = True
                for (lhsT, wkey, c) in h_chunks + x_chunks:
                    if wkey == "0x":
                        r0 = w_sb["0x"][0:54, 0:288]
                        r1 = w_sb["0x"][0:54, 288:GS]
                    else:
                        r0 = w_sb[wkey][:, c, 0:288]
                        r1 = w_sb[wkey][:, c, 288:GS]
                    nc.tensor.matmul(g0[0:rows, :], r32(lhsT), r32(r0),
                                     start=first, stop=False)
                    nc.tensor.matmul(g1[0:rows, :], r32(lhsT), r32(r1),
                                     start=first, stop=False)
                    first = False
                nc.tensor.matmul(g0[0:rows, :], ones[0:1, 0:rows],
                                 b_sb[wtag][0:1, 0:288],
                                 start=False, stop=True)
                nc.tensor.matmul(g1[0:rows, :], ones[0:1, 0:rows],
                                 b_sb[wtag][0:1, 288:GS],
                                 start=False, stop=True)

            def elementwise(g0, g1, rows, c_tile, crange):
                """gates [i f | o g]; returns h_new sbuf tile [rows, HS]"""
                sif = wk.tile([128, 2 * HS], f32, tag="sif", name="sif")
                nc.scalar.activation(sif[0:rows, :], g0[0:rows, :], AF.Sigmoid)
                so = wk.tile([128, HS], f32, tag="so", name="so")
                nc.scalar.activation(so[0:rows, :], g1[0:rows, 0:HS], AF.Sigmoid)
                tg = wk.tile([128, HS], f32, tag="tg", name="tg")
                nc.scalar.activation(tg[0:rows, :], g1[0:rows, HS:2 * HS], AF.Tanh)
                t1 = wk.tile([128, HS], f32, tag="t1", name="t1")
                cs = c_tile[crange[0]:crange[1], :]
                nc.vector.tensor_tensor(t1[0:rows, :], sif[0:rows, HS:2 * HS],
                                        cs, op=OP.mult)
                t2 = wk.tile([128, HS], f32, tag="t2", name="t2")
                nc.vector.tensor_tensor(t2[0:rows, :], sif[0:rows, 0:HS],
                                        tg[0:rows, :], op=OP.mult)
                nc.vector.tensor_tensor(cs, t1[0:rows, :], t2[0:rows, :],
                                        op=OP.add)
                tc_ = wk.tile([128, HS], f32, tag="tc", name="tc")
                nc.scalar.activation(tc_[0:rows, :], cs, AF.Tanh)
                hn = hp.tile([128, HS], f32, tag="hnew", name="hnew")
                nc.vector.tensor_tensor(hn[0:rows, :], so[0:rows, :],
                                        tc_[0:rows, :], op=OP.mult)
                return hn

            # ---- recurrence ----
            for t in range(T_OUT):
                # L0
                g0 = psg.tile([128, 288], f32, tag="g0", name="g0")
                g1 = psg.tile([128, 288], f32, tag="g1", name="g1")
                gate_mms(g0, g1, B, "0",
                         x_chunks=[(x0T[0:54, 0:B], "0x", 0)],
                         h_chunks=[(hT0[:, c, :], "0h", c) for c in range(NK)])
                hn0 = elementwise(g0, g1, B, c_st[0], (0, B))
                gin0, gout0 = allgather(HS)
                transpose_to([(gin0[:], 0, B)], hn0, B, HS)
                do_ag(gin0, gout0)
                dma_back(gout0, hT0, 0, B)

                # L1 (x = new h0)
                g0 = psg.tile([128, 288], f32, tag="g0", name="g0")
                g1 = psg.tile([128, 288], f32, tag="g1", name="g1")
                gate_mms(g0, g1, B, "1",
                         x_chunks=[(hT0[:, c, :], "1x", c) for c in range(NK)],
                         h_chunks=[(hT1[:, c, 0:B], "1h", c) for c in range(NK)])
                hn1 = elementwise(g0, g1, B, c_st[1], (0, B))
                gin1, gout1 = allgather(HS)
                transpose_to([(gin1[:], 0, B)], hn1, B, HS)
                do_ag(gin1, gout1)
                dma_back(gout1, hT1, 0, B)
                dma_back(gout1, hT1, B, 2 * B)

                # A-pair: layers 2,3 stacked (x = new h1 for BOTH)
                g0 = psg.tile([128, 288], f32, tag="g0", name="g0")
                g1 = psg.tile([128, 288], f32, tag="g1", name="g1")
                gate_mms(g0, g1, 128, "A",
                         x_chunks=[(hT1[:, c, :], "Ax", c) for c in range(NK)],
                         h_chunks=[(hTA[:, c, :], "Ah", c) for c in range(NK)])
                hnA = elementwise(g0, g1, 128, c_st["A"], (0, 128))
                gin2, gout2 = allgather(HS)
                gin3, gout3 = allgather(HS)
                transpose_to([(gin2[:], 0, B), (gin3[:], B, 2 * B)],
                             hnA, 128, HS)
                do_ag(gin2, gout2)
                do_ag(gin3, gout3)
                dma_back(gout2, hTA, 0, B)
                dma_back(gout3, hTA, B, 2 * B)

                # L4 (x = new h3)
                g0 = psg.tile([128, 288], f32, tag="g0", name="g0")
                g1 = psg.tile([128, 288], f32, tag="g1", name="g1")
                gate_mms(g0, g1, B, "L",
                         x_chunks=[(hTA[:, c, B:2 * B], "Lx", c) for c in range(NK)],
                         h_chunks=[(hTL[:, c, 0:B], "Lh", c) for c in range(NK)])
                hn4 = elementwise(g0, g1, B, c_st[4], (0, B))
                gin4, gout4 = allgather(HS)
                transpose_to([(gin4[:], 0, B)], hn4, B, HS)
                do_ag(gin4, gout4)
                dma_back(gout4, hTL, 0, B)

                # L5 (x = new h4)
                g0 = psg.tile([128, 288], f32, tag="g0", name="g0")
                g1 = psg.tile([128, 288], f32, tag="g1", name="g1")
                gate_mms(g0, g1, B, "L",
                         x_chunks=[(hTL[:, c, 0:B], "Lx", c) for c in range(NK)],
                         h_chunks=[(hTL[:, c, B:2 * B], "Lh", c) for c in range(NK)])
                hn5 = elementwise(g0, g1, B, c_st[5], (0, B))
                gin5, gout5 = allgather(HS)
                transpose_to([(gin5[:], 0, B)], hn5, B, HS)
                do_ag(gin5, gout5)
                dma_back(gout5, hTL, B, 2 * B)

                # heads (replicated on every core)
                ph = psh.tile([B, D_IN], f32, tag="ph", name="ph")
                heads = [(hTA, 0, B, 0, 12),
                         (hTA, B, 2 * B, 12, 24),
                         (hT1, 0, B, 24, 36),
                         (hTL, 0, B, 36, 45),
                         (hTL, B, 2 * B, 45, 54)]
                for src, lo, hi, olo, ohi in heads:
                    for c in range(NK):
                        nc.tensor.matmul(ph[:, olo:ohi],
                                         src[:, c, lo:hi].bitcast(f32),
                                         wh_all[:, c, olo:ohi],
                                         start=(c == 0), stop=False)
                    nc.tensor.matmul(ph[:, olo:ohi], ones[0:1, 0:B],
                                     hb1[0:1, olo:ohi],
                                     start=False, stop=True)
                pre = wk.tile([B, D_IN], f32, tag="pre", name="pre")
                nc.vector.tensor_tensor(pre[:], ph[:], x0b[:], op=OP.add)
                pre8 = wk.tile([B, D_IN], f32, tag="pre8", name="pre8")
                nc.scalar.mul(pre8[:], pre[:], 0.125)
                nc.sync.dma_start(outfull[t], pre8[:])
                if t < T_OUT - 1:
                    nc.vector.tensor_copy(x0b[:], pre[:])
                    ptq = pst.tile([128, 128], f32, tag="pt", name="pt")
                    nc.tensor.transpose(ptq[0:D_IN, 0:B], pre[0:B, 0:D_IN],
                                        ident[0:B, 0:B])
                    nc.scalar.copy(x0T[:], ptq[0:D_IN, 0:B])

            # epilogue: scatter the replicated outputs; each core keeps
            # its contiguous 1/8 of the flattened [T,B,D] buffer
            rs_out = dp.tile([NSH], f32, tag="rsout", name="rsout")
            nc.gpsimd.collective_compute(
                "ReduceScatter", OP.add, replica_groups=RG,
                ins=[outfull[:].opt()], outs=[rs_out[:].opt()])
            shf = wk.tile([100, NSH // 100], f32, tag="shf", name="shf")
            nc.sync.dma_start(shf[:],
                              rs_out[:].rearrange("(p f) -> p f", p=100))
            shb = wk.tile([100, NSH // 100], bf16, tag="shb", name="shb")
            nc.vector.tensor_copy(shb[:], shf[:])
            nc.sync.dma_start(out_d[:].rearrange("(p f) -> p f", p=100),
                              shb[:])

    nc.compile()
    return nc


def _quant(w):
    s = float(np.abs(w).max())
    if s == 0.0:
        s = 1.0
    q = np.clip(np.rint(w * (127.0 / s)), -127, 127).astype(np.int8)
    return q, np.float32(s / 127.0)


NF = NK * GS
NH = NF // 2
NL = NF // 4


def _prep_inputs(inputs):
    scales = np.zeros(24, np.float32)

    # seven big weight tensors -> int6/int5 bit-planes per core
    wqh = np.empty((NC_, 7, 128, NH), np.uint8)
    wq6l = np.empty((NC_, N6, 128, NL), np.uint8)
    wq5l = np.empty((NC_, N5, 128, _N8), np.uint8)
    i6 = i5 = 0
    for i, (tag, key) in enumerate(TAGS):
        bits = TBITS[i]
        W = inputs[key]
        s = float(np.abs(W).max())
        if s == 0.0:
            s = 1.0
        lev = 31 if bits == 6 else 15
        sq = s / lev
        hmul = 4 if bits == 6 else 2
        scales[3 * i] = hmul * sq
        scales[3 * i + 1] = -(lev + 1) * sq  # -32*s6 / -16*s5
        scales[3 * i + 2] = sq
        v = (np.clip(np.rint(W * (1.0 / sq)), -lev, lev) + lev + 1).astype(np.uint8)
        # [4g, NC_, HS, NK, 128] -> per-core flat [NC_, 128(c), NK*GS]
        v5 = v.reshape(4, NC_, HS, NK, 128)[PERM]
        flat = v5.transpose(1, 4, 3, 0, 2).reshape(NC_, 128, NF)
        if bits == 6:
            hpl = flat >> 2
            lpl = flat & 3
            wqh[:, i] = hpl[:, :, 0:NH] | (hpl[:, :, NH:] << 4)
            wq6l[:, i6] = (lpl[:, :, 0:NL] | (lpl[:, :, NL:2 * NL] << 2)
                           | (lpl[:, :, 2 * NL:3 * NL] << 4)
                           | (lpl[:, :, 3 * NL:] << 6))
            i6 += 1
        else:
            hpl = flat >> 1
            lpl = flat & 1
            wqh[:, i] = hpl[:, :, 0:NH] | (hpl[:, :, NH:] << 4)
            acc = np.zeros((NC_, 128, _N8), np.uint8)
            for j in range(8):
                acc |= (lpl[:, :, j * _N8:(j + 1) * _N8] << j).astype(np.uint8)
            wq5l[:, i5] = acc
            i5 += 1

    q, s = _quant(inputs["Wih0"])
    scales[21] = s
    w0x8 = q.reshape(4, NC_, HS, 54)[PERM].transpose(1, 3, 0, 2) \
            .reshape(NC_, 54, GS)

    whcat = np.concatenate([inputs["W_leg1"], inputs["W_leg2"],
                            inputs["W_spine"], inputs["W_arm1"],
                            inputs["W_arm2"]], axis=1).astype(np.float32)
    qh, sh = _quant(whcat)
    scales[22] = sh
    wh8 = np.ascontiguousarray(qh.reshape(NK, 128, 54))
    wscale = np.broadcast_to(scales, (128, 24)).copy()

    # biases per tag, per-core gate-col order
    b4 = np.stack([(inputs["bih" + t] + inputs["bhh" + t]).astype(np.float32)
                   for t in "01AL"])                       # [4tag, 4608]
    b4 = b4.reshape(4, 4, NC_, HS)[:, PERM]                # [tag, g', core, HS]
    b_all = np.ascontiguousarray(b4.transpose(2, 0, 1, 3).reshape(NC_, 4, GS))

    hbias = np.concatenate([inputs["b_leg1"], inputs["b_leg2"],
                            inputs["b_spine"], inputs["b_arm1"],
                            inputs["b_arm2"]]).astype(np.float32)[None, :]

    # host-side encoder means
    hs_sum = inputs["hidden_states"].sum(axis=1, dtype=np.float64)
    cin = (inputs["cell_states"].mean(axis=1, dtype=np.float64)).astype(np.float32)
    h0m = (hs_sum / T_ENC).astype(np.float32)
    h1m = ((hs_sum + inputs["global_t_state"]) / (T_ENC + 1)).astype(np.float32)
    h0T = np.ascontiguousarray(h0m.T)          # [H, B]
    h1T = np.ascontiguousarray(h1m.T)

    p0 = np.ascontiguousarray(inputs["p"][:, 0, :]).astype(np.float32)

    in_maps = []
    for j in range(NC_):
        sl = slice(j * HS, (j + 1) * HS)
        sec = {
            "wscale": wscale,
            "bias": b_all[j],
            "hb1": hbias,
            "h0T": h0T[sl],
            "h1T": h1T[sl],
            "cin": cin[:, sl],
            "p0": p0,
            "wh8": wh8,
            "w0x8": w0x8[j],
            "wqh": wqh[j],
            "wq6l": wq6l[j],
            "wq5l": wq5l[j],
        }
        bl = np.empty(BLOB_BYTES, np.uint8)
        for name, arr in sec.items():
            off, nb = BLOB_OFF[name]
            flat = np.ascontiguousarray(arr).view(np.uint8).reshape(-1)
            assert flat.nbytes == nb, (name, flat.nbytes, nb)
            bl[off:off + nb] = flat
        in_maps.append({"blob": bl})
    return in_maps


def kernel(**inputs):
    global _compiled
    import concourse.bass_utils as bass_utils
    if _compiled is None:
        _compiled = _build()
    in_maps = _prep_inputs(inputs)
    res = bass_utils.run_bass_kernel_spmd(
        _compiled, in_maps, core_ids=list(range(NC_)))
    flat = np.concatenate([np.asarray(res.results[c]["out"])
                           for c in range(NC_)]).astype(np.float32)
    return np.ascontiguousarray(
        flat.reshape(T_OUT, B, D_IN).transpose(1, 0, 2))


# revision 8
# speedup vs baseline: 2.5756x; 1.0001x over previous
"""Kinematics LSTM decoder on 8 trn2 NeuronCores — wire-optimized.

The axon tunnel moves host->device bytes at ~25-45MB/s and device exec is
~6ms, so the per-call wall is dominated by input transfer plus a ~0.9s
per-call XLA+walrus recompile. Optimizations vs the 186MB/8.6s baseline:

- big LSTM weights shipped as packed int6 (Wih1/Whh1) / int5
  (Whh0 + A/L cells) bit-planes, unpacked+dequantized on device into
  SBUF-resident f32r tiles (real-input l2rel 8.8e-3, maxrel 1.19e-2 vs
  the 2e-2 gate); Wih0 + output heads as int8
- encoder means (h0/h1/c_init) computed host-side; zeros/identity
  generated on device; head bias folded into the head-matmul PSUM group
- ALL per-core inputs packed into ONE uint8 blob (~3.6MB/core, 28.6MB
  total) so the tunnel sees a single large array; output in bf16
- jax persistent compilation cache enabled so warm calls skip the
  per-call NEFF recompile

Device strategy (unchanged from the original baseline): model-parallel
over the 4608 gate dim (576 gate cols / core = 144 h cols / core).
Recurrence: 25 steps x 6 cells; per-cell AllGather of the transposed h
slice through DRAM bounce buffers. Gates layout [batch, gatecols],
per-core col order [i f | o g]; matmuls in f32r. Layers 2,3 share
weights AND input -> batch-stacked (M=128).
"""
import numpy as np

B, T_ENC, D_IN, H, T_OUT = 64, 49, 54, 1152, 25
NC_ = 8          # cores
HS = H // NC_    # 144 h cols per core
GS = 4 * HS      # 576 gate cols per core
NK = H // 128    # 9 contraction chunks

PERM = [0, 1, 3, 2]  # pytorch gate order (i,f,g,o) -> per-core col order (i,f,o,g)
TAGS = [("0h", "Whh0"), ("1x", "Wih1"), ("1h", "Whh1"),
        ("Ax", "WihA"), ("Ah", "WhhA"), ("Lx", "WihL"), ("Lh", "WhhL")]
TAGIDX = {"0": 0, "1": 1, "A": 2, "L": 3}

_NF = NK * GS
_NH = _NF // 2
_NL = _NF // 4
_N8 = _NF // 8

# bits per big-weight tensor, in TAGS order (1x,1h at 6; 0h + A/L cells at 5 —
# real-input sim: l2rel 8.7e-3, maxrel 1.19e-2 vs the 2e-2 gate)
TBITS = [5, 6, 6, 5, 5, 5, 5]
N6 = sum(1 for b in TBITS if b == 6)
N5 = sum(1 for b in TBITS if b == 5)


def _blob_layout():
    layout = [
        ("wscale", 128 * 24 * 4),
        ("bias", 4 * GS * 4),
        ("hb1", D_IN * 4),
        ("h0T", HS * B * 4),
        ("h1T", HS * B * 4),
        ("cin", B * HS * 4),
        ("p0", B * D_IN * 4),
        ("wh8", NK * 128 * 54),
        ("w0x8", 54 * GS),
        ("wqh", 7 * 128 * _NH),
        ("wq6l", N6 * 128 * _NL),
        ("wq5l", N5 * 128 * _N8),
    ]
    off, d = 0, {}
    for name, nb in layout:
        d[name] = (off, nb)
        off += nb
    return d, off


BLOB_OFF, BLOB_BYTES = _blob_layout()

_compiled = None


def _enable_jax_cache():
    """Persistent XLA executable cache: without it every
    run_bass_kernel_spmd call re-lowers + re-runs the walrus NEFF
    compile (~0.9s/call)."""
    try:
        import jax
        jax.config.update("jax_compilation_cache_dir", "/tmp/bass_jax_cache")
        jax.config.update("jax_persistent_cache_min_entry_size_bytes", -1)
        jax.config.update("jax_persistent_cache_min_compile_time_secs", 0)
    except Exception:
        pass


_enable_jax_cache()



def _build():
    import concourse.bass as bass
    import concourse.bacc as bacc
    import concourse.tile as tile
    import concourse.mybir as mybir

    f32 = mybir.dt.float32
    f32r = mybir.dt.float32r
    bf16 = mybir.dt.bfloat16
    i8 = mybir.dt.int8
    u8 = mybir.dt.uint8
    AF = mybir.ActivationFunctionType
    OP = mybir.AluOpType

    NF = NK * GS          # 5184 flat weight cols per partition
    NH = NF // 2          # 2592
    NL = NF // 4          # 1296

    nc = bacc.Bacc("TRN2", target_bir_lowering=False, debug=False,
                   num_devices=NC_)

    # single per-core input blob; section offsets must match _prep_inputs
    blob = nc.dram_tensor("blob", [BLOB_BYTES], u8, kind="ExternalInput")

    def bsec(name, dt_):
        off, nbytes = BLOB_OFF[name]
        ap = blob[off:off + nbytes]
        return ap if dt_ == u8 else ap.bitcast(dt_)

    # Every core computes bit-identical outputs; ReduceScatter of pre/8
    # leaves each core exactly its 1/8 shard, cutting the donated-zeros
    # upload and the 8-core result fetch from 1.38MB to 173KB each way.
    NOUT = T_OUT * B * D_IN          # 86400
    NSH = NOUT // NC_                # 10800 elements per core
    out_d = nc.dram_tensor("out", [NSH], bf16, kind="ExternalOutput")

    RG = [list(range(NC_))]

    with tile.TileContext(nc) as tc:
        with tc.tile_pool(name="wpool", bufs=1) as wp, \
             tc.tile_pool(name="stg", bufs=1) as stg, \
             tc.tile_pool(name="state", bufs=1) as st, \
             tc.tile_pool(name="work", bufs=3) as wk, \
             tc.tile_pool(name="hnewp", bufs=2) as hp, \
             tc.tile_pool(name="psg", bufs=2, space="PSUM") as psg, \
             tc.tile_pool(name="pst", bufs=2, space="PSUM") as pst, \
             tc.tile_pool(name="psh", bufs=1, space="PSUM") as psh, \
             tc.tile_pool(name="dram", bufs=6, space="DRAM") as dp:

            # ---- scales / misc constants ----
            wsc = wp.tile([128, 24], f32, tag="wsc", name="wsc")
            nc.sync.dma_start(
                wsc[:], bsec("wscale", f32).rearrange("(p f) -> p f", p=128))
            ones = wp.tile([1, 128], f32, tag="ones", name="ones")
            nc.vector.memset(ones[:], 1.0)
            ones128 = wp.tile([128, 128], f32, tag="ones128", name="ones128")
            nc.vector.memset(ones128[:], 1.0)
            ident = wp.tile([128, 128], f32, tag="ident", name="ident")
            nc.gpsimd.affine_select(ident[:], ones128[:], pattern=[[-1, 128]],
                                    compare_op=OP.is_equal, fill=0.0,
                                    base=0, channel_multiplier=1)
            hb1 = wp.tile([1, D_IN], f32, tag="hb1", name="hb1")
            nc.sync.dma_start(
                hb1[:], bsec("hb1", f32).rearrange("(p f) -> p f", p=1))
            boff = BLOB_OFF["bias"][0]
            b_sb = {}
            for ti, tg in enumerate("01AL"):
                b_sb[tg] = wp.tile([1, GS], f32, tag="b" + tg, name="b" + tg)
                nc.sync.dma_start(
                    b_sb[tg][:],
                    blob[boff + ti * GS * 4:boff + (ti + 1) * GS * 4]
                    .bitcast(f32).rearrange("(p f) -> p f", p=1))

            # ---- unpack int6/int5 weights into SBUF-resident f32r tiles ----
            # int6: v = round(w/s)+32 in [1,63]; h=v>>2 (4b), l=v&3 (2b);
            #       w = h*(4s) - 32s + l*s
            # int5: v = round(w/s)+16 in [1,31]; h=v>>1 (4b), l=v&1 (1b);
            #       w = h*(2s) - 16s + l*s
            N8 = NF // 8
            w_sb = {}
            hoff = BLOB_OFF["wqh"][0]
            l6off = BLOB_OFF["wq6l"][0]
            l5off = BLOB_OFF["wq5l"][0]
            i6 = i5 = 0
            for i, (tag, _) in enumerate(TAGS):
                bits = TBITS[i]
                sh = stg.tile([128, NH], u8, tag="sth", name="sth" + tag)
                nc.sync.dma_start(
                    sh[:], blob[hoff + i * 128 * NH:hoff + (i + 1) * 128 * NH]
                    .rearrange("(p f) -> p f", p=128))
                if bits == 6:
                    lsz = NL
                    loff = l6off + i6 * 128 * NL
                    i6 += 1
                else:
                    lsz = N8
                    loff = l5off + i5 * 128 * N8
                    i5 += 1
                sl = stg.tile([128, lsz], u8, tag=f"stl{bits}", name="stl" + tag)
                nc.sync.dma_start(
                    sl[:], blob[loff:loff + 128 * lsz]
                    .rearrange("(p f) -> p f", p=128))
                w_sb[tag] = wp.tile([128, NK, GS], f32r, tag="w" + tag, name="w" + tag)
                wf = w_sb[tag][:].rearrange("p a b -> p (a b)")
                shi = wsc[:, 3 * i:3 * i + 1]
                soff = wsc[:, 3 * i + 1:3 * i + 2]
                s1 = wsc[:, 3 * i + 2:3 * i + 3]
                # hi plane (4 bits either way)
                hsh = 2 if bits == 6 else 1
                th = stg.tile([128, NH], u8, tag="th", name="th" + tag)
                nc.vector.tensor_scalar(th[:], sh[:], 15, None, op0=OP.bitwise_and)
                nc.vector.tensor_scalar(wf[:, 0:NH], th[:], shi, soff,
                                        op0=OP.mult, op1=OP.add)
                nc.vector.tensor_scalar(th[:], sh[:], 4, None,
                                        op0=OP.logical_shift_right)
                nc.vector.tensor_scalar(wf[:, NH:NF], th[:], shi, soff,
                                        op0=OP.mult, op1=OP.add)
                # lo plane: 8 subranges of N8 columns each
                for k in range(8):
                    if bits == 6:
                        qd, hh = k // 2, k % 2
                        lsrc = sl[:, hh * N8:(hh + 1) * N8]
                        shift, mask, last = 2 * qd, 3, (qd == 3)
                    else:
                        lsrc = sl[:]
                        shift, mask, last = k, 1, (k == 7)
                    dst = wf[:, k * N8:(k + 1) * N8] if bits == 5 else \
                        wf[:, (k // 2) * NL + (k % 2) * N8:
                           (k // 2) * NL + (k % 2 + 1) * N8]
                    tl = stg.tile([128, N8], u8, tag="tl", name="tl")
                    if shift == 0:
                        nc.vector.tensor_scalar(tl[:], lsrc, mask, None,
                                                op0=OP.bitwise_and)
                    elif last:
                        nc.vector.tensor_scalar(tl[:], lsrc, shift, None,
                                                op0=OP.logical_shift_right)
                    else:
                        nc.vector.tensor_scalar(tl[:], lsrc, shift, mask,
                                                op0=OP.logical_shift_right,
                                                op1=OP.bitwise_and)
                    tf = stg.tile([128, N8], f32, tag="tf", name="tf")
                    nc.vector.tensor_scalar(tf[:], tl[:], s1, None,
                                            op0=OP.mult)
                    nc.vector.tensor_tensor(dst, dst, tf[:], op=OP.add)

            sg0 = stg.tile([54, GS], i8, tag="stg0", name="stg0x")
            nc.sync.dma_start(
                sg0[:], bsec("w0x8", i8).rearrange("(p f) -> p f", p=54))
            w_sb["0x"] = wp.tile([54, GS], f32r, tag="w0x", name="w0x")
            nc.vector.tensor_scalar(w_sb["0x"][:], sg0[:], wsc[0:54, 21:22],
                                    None, op0=OP.mult)
            sgh = stg.tile([128, NK, 54], i8, tag="stgh", name="stgh")
            nc.sync.dma_start(
                sgh[:], bsec("wh8", i8).rearrange("(c k n) -> k c n",
                                                  c=NK, k=128, n=54))
            wh_all = wp.tile([128, NK, 54], f32, tag="whall", name="whall")
            nc.vector.tensor_scalar(wh_all[:], sgh[:], wsc[:, 22:23],
                                    None, op0=OP.mult)

            # ---- persistent state ----
            hT0 = st.tile([128, NK, B], f32r, tag="hT0", name="hT0")        # h0.T
            hT1 = st.tile([128, NK, 2 * B], f32r, tag="hT1", name="hT1")    # h1.T duplicated
            hTA = st.tile([128, NK, 2 * B], f32r, tag="hTA", name="hTA")    # h2.T | h3.T
            hTL = st.tile([128, NK, 2 * B], f32r, tag="hTL", name="hTL")    # h4.T | h5.T
            c_st = {0: st.tile([B, HS], f32, tag="c0", name="c0"),
                    1: st.tile([B, HS], f32, tag="c1", name="c1"),
                    "A": st.tile([2 * B, HS], f32, tag="cA", name="cA"),
                    4: st.tile([B, HS], f32, tag="c4", name="c4"),
                    5: st.tile([B, HS], f32, tag="c5", name="c5")}
            x0b = st.tile([B, D_IN], f32, tag="x0b", name="x0b")
            x0T = st.tile([D_IN, B], f32r, tag="x0T", name="x0T")

            zf = wp.tile([128, 2 * B], f32, tag="zf", name="zf")
            nc.vector.memset(zf[:], 0.0)
            for c in range(NK):
                nc.scalar.copy(hTA[:, c, :], zf[:])
                nc.scalar.copy(hTL[:, c, :], zf[:])
            nc.vector.memset(c_st["A"][:], 0.0)
            nc.vector.memset(c_st[4][:], 0.0)
            nc.vector.memset(c_st[5][:], 0.0)

            r32 = lambda ap: ap.bitcast(f32r)

            def transpose_to(dst_dram_slices, src_sb, rows, cols):
                """src_sb [rows<=128, cols] -> transposed [cols, rows] written
                to dram col-splits."""
                done = 0
                while done < cols:
                    n = min(128, cols - done)
                    pt = pst.tile([128, 128], f32, tag="pt", name="pt")
                    nc.tensor.transpose(pt[0:n, 0:rows],
                                        src_sb[0:rows, done:done + n],
                                        ident[0:rows, 0:rows])
                    cp = wk.tile([128, 128], f32r, tag="tcp", name="tcp")
                    nc.scalar.copy(cp[0:n, 0:rows], pt[0:n, 0:rows])
                    for (dap, lo, hi) in dst_dram_slices:
                        nc.sync.dma_start(dap[done:done + n, :],
                                          cp[0:n, lo:hi])
                    done += n

            def allgather(n_rows):
                gin = dp.tile([n_rows, B], f32r, tag="agin", name="agin")
                gout = dp.tile([NC_ * n_rows, B], f32r, tag="agout", name="agout")
                return gin, gout

            def do_ag(gin, gout):
                nc.gpsimd.collective_compute(
                    "AllGather", OP.bypass, replica_groups=RG,
                    ins=[gin[:].opt()], outs=[gout[:].opt()])

            def dma_back(gout, dst, lo, hi):
                nc.sync.dma_start(
                    dst[:, :, lo:hi],
                    gout[:].rearrange("(c k) n -> k c n", k=128))

            # ---- prologue: states from host-computed means ----
            nc.sync.dma_start(
                c_st[0][:], bsec("cin", f32).rearrange("(p f) -> p f", p=B))
            nc.vector.tensor_copy(c_st[1][:], c_st[0][:])

            for (nm, dsts) in (("h0T", [(hT0, 0, B)]),
                               ("h1T", [(hT1, 0, B), (hT1, B, 2 * B)])):
                gin, gout = allgather(HS)
                nc.sync.dma_start(
                    gin[:], bsec(nm, f32r).rearrange("(p f) -> p f", p=HS))
                do_ag(gin, gout)
                for (dst, lo, hi) in dsts:
                    dma_back(gout, dst, lo, hi)

            # x0
            nc.sync.dma_start(
                x0b[:], bsec("p0", f32).rearrange("(p f) -> p f", p=B))

            # staging buffer for all 25 steps' outputs (f32, pre-scaled by
            # 1/8 so the final ReduceScatter(add) over 8 identical copies
            # reconstructs the value)
            outfull = dp.tile([T_OUT, B, D_IN], f32, tag="outfull",
                              name="outfull")
            ptp = pst.tile([128, 128], f32, tag="pt", name="pt")
            nc.tensor.transpose(ptp[0:D_IN, 0:B], x0b[0:B, 0:D_IN],
                                ident[0:B, 0:B])
            nc.scalar.copy(x0T[:], ptp[0:D_IN, 0:B])

            # ---- helpers for the recurrence ----
            def gate_mms(g0, g1, rows, wtag, x_chunks, h_chunks):
                first = True
                for (lhsT, wkey, c) in h_chunks + x_chunks:
                    if wkey == "0x":
                        r0 = w_sb["0x"][0:54, 0:288]
                        r1 = w_sb["0x"][0:54, 288:GS]
                    else:
                        r0 = w_sb[wkey][:, c, 0:288]
                        r1 = w_sb[wkey][:, c, 288:GS]
                    nc.tensor.matmul(g0[0:rows, :], r32(lhsT), r32(r0),
                                     start=first, stop=False)
                    nc.tensor.matmul(g1[0:rows, :], r32(lhsT), r32(r1),
                                     start=first, stop=False)
                    first = False
                nc.tensor.matmul(g0[0:rows, :], ones[0:1, 0:rows],
                                 b_sb[wtag][0:1, 0:288],
                                 start=False, stop=True)
                nc.tensor.matmul(g1[0:rows, :], ones[0:1, 0:rows],
                                 b_sb[wtag][0:1, 288:GS],
                                 start=False, stop=True)

            def elementwise(g0, g1, rows, c_tile, crange):
                """gates [i f | o g]; returns h_new sbuf tile [rows, HS]"""
                sif = wk.tile([128, 2 * HS], f32, tag="sif", name="sif")
                nc.scalar.activation(sif[0:rows, :], g0[0:rows, :], AF.Sigmoid)
                so = wk.tile([128, HS], f32, tag="so", name="so")
                nc.scalar.activation(so[0:rows, :], g1[0:rows, 0:HS], AF.Sigmoid)
                tg = wk.tile([128, HS], f32, tag="tg", name="tg")
                nc.scalar.activation(tg[0:rows, :], g1[0:rows, HS:2 * HS], AF.Tanh)
                t1 = wk.tile([128, HS], f32, tag="t1", name="t1")
                cs = c_tile[crange[0]:crange[1], :]
                nc.vector.tensor_tensor(t1[0:rows, :], sif[0:rows, HS:2 * HS],
                                        cs, op=OP.mult)
                t2 = wk.tile([128, HS], f32, tag="t2", name="t2")
                nc.vector.tensor_tensor(t2[0:rows, :], sif[0:rows, 0:HS],
                                        tg[0:rows, :], op=OP.mult)
                nc.vector.tensor_tensor(cs, t1[0:rows, :], t2[0:rows, :],
                                        op=OP.add)
                tc_ = wk.tile([128, HS], f32, tag="tc", name="tc")
                nc.scalar.activation(tc_[0:rows, :], cs, AF.Tanh)
                hn = hp.tile([128, HS], f32, tag="hnew", name="hnew")
                nc.vector.tensor_tensor(hn[0:rows, :], so[0:rows, :],
                                        tc_[0:rows, :], op=OP.mult)
                return hn

            # ---- recurrence ----
            for t in range(T_OUT):
                # L0
                g0 = psg.tile([128, 288], f32, tag="g0", name="g0")
                g1 = psg.tile([128, 288], f32, tag="g1", name="g1")
                gate_mms(g0, g1, B, "0",
                         x_chunks=[(x0T[0:54, 0:B], "0x", 0)],
                         h_chunks=[(hT0[:, c, :], "0h", c) for c in range(NK)])
                hn0 = elementwise(g0, g1, B, c_st[0], (0, B))
                gin0, gout0 = allgather(HS)
                transpose_to([(gin0[:], 0, B)], hn0, B, HS)
                do_ag(gin0, gout0)
                dma_back(gout0, hT0, 0, B)

                # L1 (x = new h0)
                g0 = psg.tile([128, 288], f32, tag="g0", name="g0")
                g1 = psg.tile([128, 288], f32, tag="g1", name="g1")
                gate_mms(g0, g1, B, "1",
                         x_chunks=[(hT0[:, c, :], "1x", c) for c in range(NK)],
                         h_chunks=[(hT1[:, c, 0:B], "1h", c) for c in range(NK)])
                hn1 = elementwise(g0, g1, B, c_st[1], (0, B))
                gin1, gout1 = allgather(HS)
                transpose_to([(gin1[:], 0, B)], hn1, B, HS)
                do_ag(gin1, gout1)
                dma_back(gout1, hT1, 0, B)
                dma_back(gout1, hT1, B, 2 * B)

                # A-pair: layers 2,3 stacked (x = new h1 for BOTH)
                g0 = psg.tile([128, 288], f32, tag="g0", name="g0")
                g1 = psg.tile([128, 288], f32, tag="g1", name="g1")
                gate_mms(g0, g1, 128, "A",
                         x_chunks=[(hT1[:, c, :], "Ax", c) for c in range(NK)],
                         h_chunks=[(hTA[:, c, :], "Ah", c) for c in range(NK)])
                hnA = elementwise(g0, g1, 128, c_st["A"], (0, 128))
                gin2, gout2 = allgather(HS)
                gin3, gout3 = allgather(HS)
                transpose_to([(gin2[:], 0, B), (gin3[:], B, 2 * B)],
                             hnA, 128, HS)
                do_ag(gin2, gout2)
                do_ag(gin3, gout3)
                dma_back(gout2, hTA, 0, B)
                dma_back(gout3, hTA, B, 2 * B)

                # L4 (x = new h3)
                g0 = psg.tile([128, 288], f32, tag="g0", name="g0")
                g1 = psg.tile([128, 288], f32, tag="g1", name="g1")
                gate_mms(g0, g1, B, "L",
                         x_chunks=[(hTA[:, c, B:2 * B], "Lx", c) for c in range(NK)],
                         h_chunks=[(hTL[:, c, 0:B], "Lh", c) for c in range(NK)])
                hn4 = elementwise(g0, g1, B, c_st[4], (0, B))
                gin4, gout4 = allgather(HS)
                transpose_to([(gin4[:], 0, B)], hn4, B, HS)
                do_ag(gin4, gout4)
                dma_back(gout4, hTL, 0, B)

                # L5 (x = new h4)
                g0 = psg.tile([128, 288], f32, tag="g0", name="g0")
                g1 = psg.tile([128, 288], f32, tag="g1", name="g1")
                gate_mms(g0, g1, B, "L",
                         x_chunks=[(hTL[:, c, 0:B], "Lx", c) for c in range(NK)],
                         h_chunks=[(hTL[:, c, B:2 * B], "Lh", c) for c in range(NK)])
                hn5 = elementwise(g0, g1, B, c_st[5], (0, B))
                gin5, gout5 = allgather(HS)
                transpose_to([(gin5[:], 0, B)], hn5, B, HS)
                do_ag(gin5, gout5)
                dma_back(gout5, hTL, B, 2 * B)

                # heads (replicated on every core)
                ph = psh.tile([B, D_IN], f32, tag="ph", name="ph")
                heads = [(hTA, 0, B, 0, 12),
                         (hTA, B, 2 * B, 12, 24),
                         (hT1, 0, B, 24, 36),
                         (hTL, 0, B, 36, 45),
                         (hTL, B, 2 * B, 45, 54)]
                for src, lo, hi, olo, ohi in heads:
                    for c in range(NK):
                        nc.tensor.matmul(ph[:, olo:ohi],
                                         src[:, c, lo:hi].bitcast(f32),
                                         wh_all[:, c, olo:ohi],
                                         start=(c == 0), stop=False)
                    nc.tensor.matmul(ph[:, olo:ohi], ones[0:1, 0:B],
                                     hb1[0:1, olo:ohi],
                                     start=False, stop=True)
                pre = wk.tile([B, D_IN], f32, tag="pre", name="pre")
                nc.vector.tensor_tensor(pre[:], ph[:], x0b[:], op=OP.add)
                pre8 = wk.tile([B, D_IN], f32, tag="pre8", name="pre8")
                nc.scalar.mul(pre8[:], pre[:], 0.125)
                nc.sync.dma_start(outfull[t], pre8[:])
                if t < T_OUT - 1:
                    nc.vector.tensor_copy(x0b[:], pre[:])
                    ptq = pst.tile([128, 128], f32, tag="pt", name="pt")
                    nc.tensor.transpose(ptq[0:D_IN, 0:B], pre[0:B, 0:D_IN],
                                        ident[0:B, 0:B])
                    nc.scalar.copy(x0T[:], ptq[0:D_IN, 0:B])

            # epilogue: scatter the replicated outputs; each core keeps
            # its contiguous 1/8 of the flattened [T,B,D] buffer
            rs_out = dp.tile([NSH], f32, tag="rsout", name="rsout")
            nc.gpsimd.collective_compute(
                "ReduceScatter", OP.add, replica_groups=RG,
                ins=[outfull[:].opt()], outs=[rs_out[:].opt()])
            shf = wk.tile([100, NSH // 100], f32, tag="shf", name="shf")
            nc.sync.dma_start(shf[:],
                              rs_out[:].rearrange("(p f) -> p f", p=100))
            shb = wk.tile([100, NSH // 100], bf16, tag="shb", name="shb")
            nc.vector.tensor_copy(shb[:], shf[:])
            nc.sync.dma_start(out_d[:].rearrange("(p f) -> p f", p=100),
                              shb[:])

    nc.compile()
    return nc


def _quant(w):
    s = float(np.abs(w).max())
    if s == 0.0:
        s = 1.0
    q = np.clip(np.rint(w * (127.0 / s)), -127, 127).astype(np.int8)
    return q, np.float32(s / 127.0)


NF = NK * GS
NH = NF // 2
NL = NF // 4


def _prep_inputs(inputs):
    scales = np.zeros(24, np.float32)

    # seven big weight tensors -> int6/int5 bit-planes per core
    wqh = np.empty((NC_, 7, 128, NH), np.uint8)
    wq6l = np.empty((NC_, N6, 128, NL), np.uint8)
    wq5l = np.empty((NC_, N5, 128, _N8), np.uint8)
    i6 = i5 = 0
    for i, (tag, key) in enumerate(TAGS):
        bits = TBITS[i]
        W = inputs[key]
        s = float(np.abs(W).max())
        if s == 0.0:
            s = 1.0
        lev = 31 if bits == 6 else 15
        sq = s / lev
        hmul = 4 if bits == 6 else 2
        scales[3 * i] = hmul * sq
        scales[3 * i + 1] = -(lev + 1) * sq  # -32*s6 / -16*s5
        scales[3 * i + 2] = sq
        v = (np.clip(np.rint(W * (1.0 / sq)), -lev, lev) + lev + 1).astype(np.uint8)
        # [4g, NC_, HS, NK, 128] -> per-core flat [NC_, 128(c), NK*GS]
        v5 = v.reshape(4, NC_, HS, NK, 128)[PERM]
        flat = v5.transpose(1, 4, 3, 0, 2).reshape(NC_, 128, NF)
        if bits == 6:
            hpl = flat >> 2
            lpl = flat & 3
            wqh[:, i] = hpl[:, :, 0:NH] | (hpl[:, :, NH:] << 4)
            wq6l[:, i6] = (lpl[:, :, 0:NL] | (lpl[:, :, NL:2 * NL] << 2)
                           | (lpl[:, :, 2 * NL:3 * NL] << 4)
                           | (lpl[:, :, 3 * NL:] << 6))
            i6 += 1
        else:
            hpl = flat >> 1
            lpl = flat & 1
            wqh[:, i] = hpl[:, :, 0:NH] | (hpl[:, :, NH:] << 4)
            acc = np.zeros((NC_, 128, _N8), np.uint8)
            for j in range(8):
                acc |= (lpl[:, :, j * _N8:(j + 1) * _N8] << j).astype(np.uint8)
            wq5l[:, i5] = acc
            i5 += 1

    q, s = _quant(inputs["Wih0"])
    scales[21] = s
    w0x8 = q.reshape(4, NC_, HS, 54)[PERM].transpose(1, 3, 0, 2) \
            .reshape(NC_, 54, GS)

    whcat = np.concatenate([inputs["W_leg1"], inputs["W_leg2"],
                            inputs["W_spine"], inputs["W_arm1"],
                            inputs["W_arm2"]], axis=1).astype(np.float32)
    qh, sh = _quant(whcat)
    scales[22] = sh
    wh8 = np.ascontiguousarray(qh.reshape(NK, 128, 54))
    wscale = np.broadcast_to(scales, (128, 24)).copy()

    # biases per tag, per-core gate-col order
    b4 = np.stack([(inputs["bih" + t] + inputs["bhh" + t]).astype(np.float32)
                   for t in "01AL"])                       # [4tag, 4608]
    b4 = b4.reshape(4, 4, NC_, HS)[:, PERM]                # [tag, g', core, HS]
    b_all = np.ascontiguousarray(b4.transpose(2, 0, 1, 3).reshape(NC_, 4, GS))

    hbias = np.concatenate([inputs["b_leg1"], inputs["b_leg2"],
                            inputs["b_spine"], inputs["b_arm1"],
                            inputs["b_arm2"]]).astype(np.float32)[None, :]

    # host-side encoder means
    hs_sum = inputs["hidden_states"].sum(axis=1, dtype=np.float64)
    cin = (inputs["cell_states"].mean(axis=1, dtype=np.float64)).astype(np.float32)
    h0m = (hs_sum / T_ENC).astype(np.float32)
    h1m = ((hs_sum + inputs["global_t_state"]) / (T_ENC + 1)).astype(np.float32)
    h0T = np.ascontiguousarray(h0m.T)          # [H, B]
    h1T = np.ascontiguousarray(h1m.T)

    p0 = np.ascontiguousarray(inputs["p"][:, 0, :]).astype(np.float32)

    in_maps = []
    for j in range(NC_):
        sl = slice(j * HS, (j + 1) * HS)
        sec = {
            "wscale": wscale,
            "bias": b_all[j],
            "hb1": hbias,
            "h0T": h0T[sl],
            "h1T": h1T[sl],
            "cin": cin[:, sl],
            "p0": p0,
            "wh8": wh8,
            "w0x8": w0x8[j],
            "wqh": wqh[j],
            "wq6l": wq6l[j],
            "wq5l": wq5l[j],
        }
        bl = np.empty(BLOB_BYTES, np.uint8)
        for name, arr in sec.items():
            off, nb = BLOB_OFF[name]
            flat = np.ascontiguousarray(arr).view(np.uint8).reshape(-1)
            assert flat.nbytes == nb, (name, flat.nbytes, nb)
            bl[off:off + nb] = flat
        in_maps.append({"blob": bl})
    return in_maps


def kernel(**inputs):
    global _compiled
    import concourse.bass_utils as bass_utils
    if _compiled is None:
        _compiled = _build()
    in_maps = _prep_inputs(inputs)
    res = bass_utils.run_bass_kernel_spmd(
        _compiled, in_maps, core_ids=list(range(NC_)))
    flat = np.concatenate([np.asarray(res.results[c]["out"])
                           for c in range(NC_)]).astype(np.float32)
    return np.ascontiguousarray(
        flat.reshape(T_OUT, B, D_IN).transpose(1, 0, 2))
